# revision 1
# baseline (speedup 1.0000x reference)
"""Linear attention (B=4, S=4096, D=1024, H=16) on 8 TRN2 NeuronCores.

Sharding: core = (batch, head-half): each core handles one batch's 8 heads.
 - x is host-transposed to xT [D, S] per batch so both operand orientations
   of every matmul come out of the tensor engine with no on-device transpose.
 - Wqkv column-sharded per head-half; Wo row-sharded; host sums the two
   partial y's per batch (row-parallel unshard).

Two-phase dataflow (all matmuls bf16, fp32 PSUM accumulate):

phase 1 (per 512-token block): K,V projection token-major (lhsT=xT slice,
  rhs=Wk/Wv) -> elu+1(K) -> [KV | K_sum^T] PSUM accumulation per head-pair
  (vst carries a ones column so one matmul does both). Q is NOT computed
  here -- it is deferred to phase 2 so the PE has independent work to chew
  on across the KV -> attention transition (no pipeline bubble), and so x
  (kept resident in SBUF, 8MB bf16) is the only phase-1 input.
  Block 0 runs k-outer (4 simultaneous PSUM chains, one per 128-token
  subtile) so compute starts as soon as the first (wkv, x) DMA chunk lands
  instead of waiting for the full weight load.

phase 2 (per block, software-pipelined across j):
  QT [512f, 512s] feature-major (lhsT=Wq, rhs=xT slice) -> elu+1 -> bf16
  psc[128,s] = blockdiag(KV_h0, KV_h1)^T @ QT_pair: both heads of a pair
    in one matmul; ACT-evicted to outu
  norm: lhsT = [ksum_h0 replicated x64 | ksum_h1 replicated x64] so the
    matmul output IS the normalizer broadcast across all 128 partitions
    (no separate broadcast matmul); 1/x via the single-instruction DVE
    fast reciprocal (no Ln/Exp ACT ops, no activation-table switches)
  outT = outu * rcp (one DVE mult per pair, bf16)
  y[s,:] = outT^T @ Wo per 128-token subtile, fp32 out, DMAed per subtile
    (512KB chunks) to keep the drain tail short.
"""

import numpy as np

import concourse.bacc as bacc
import concourse.mybir as mybir
import concourse.tile as tile
from concourse.bass_utils import run_bass_kernel_spmd

F32 = mybir.dt.float32
BF16 = mybir.dt.bfloat16
ACT = mybir.ActivationFunctionType

P = 128
B, S, D = 4, 4096, 1024
H = 16
HD = 64

FSH = 512            # features per core for each of Q, K, V (8 heads)
KSUB = D // P        # 8 contraction subtiles
SBLK = 512           # tokens per block
NBLK = S // SBLK     # 8 blocks
TSUB = SBLK // P     # 4 token subtiles per block
NPAIR = 4            # head pairs per core

_NC_CACHE = None


def build():
    nc = bacc.Bacc(target_bir_lowering=False)
    xT = nc.dram_tensor("xT", [D, S], BF16, kind="ExternalInput")
    wqkv = nc.dram_tensor("wqkv", [D, 3 * FSH], BF16, kind="ExternalInput")
    wo = nc.dram_tensor("wo", [FSH, D], BF16, kind="ExternalInput")
    y = nc.dram_tensor("y", [S, D], F32, kind="ExternalOutput")

    xT_r = xT.rearrange("(ko p) s -> p ko s", p=P)        # [128, 8, 4096]
    wqkv_r = wqkv.rearrange("(ko p) f -> p ko f", p=P)    # [128, 8, 1536]
    wo_r = wo.rearrange("(fo p) n -> p fo n", p=P)        # [128, 4, 1024]
    y_rt = y.rearrange(
        "(j t p) (nh n) -> j t nh p n", t=TSUB, p=P, nh=2
    )  # [8,4,2,128,512]

    with tile.TileContext(nc) as tc:
        import contextlib

        with contextlib.ExitStack() as ctx:
            wpool = ctx.enter_context(tc.tile_pool(name="wpool", bufs=1))

            # persistent SBUF
            xt_sb = wpool.tile([P, KSUB, S], BF16)          # all of x, 64KB/p
            wqkv_sb = wpool.tile([P, KSUB, 3 * FSH], BF16)  # [wq|wk|wv]
            wo_sb = wpool.tile([P, FSH // P, D], BF16)
            # per-pair block-diagonal [[KV_h0, 0], [0, KV_h1]] (128x128)
            lhsT2_sb = [
                wpool.tile([P, P], BF16, name=f"l2{p}") for p in range(NPAIR)
            ]
            # per-pair [ksum_h0 x64 | ksum_h1 x64] replicated along free dim:
            # norm matmul output comes out already broadcast per head-half
            ksumrep_sb = [
                wpool.tile([P, P], BF16, name=f"kr{p}") for p in range(NPAIR)
            ]

            # Only block-0-critical transfers go on the sync queue (issued at
            # t=0): x block 0 + wv interleaved per contraction subtile, then
            # wk. Everything else (x blocks 1-7, wq, wo) is issued from the
            # scalar engine's DGE mid-block-0 — gated behind ops that already
            # depend on block-0 data — so it cannot steal HBM bandwidth from
            # the startup-critical path.
            for k in range(KSUB):
                nc.sync.dma_start(
                    out=xt_sb[:, k, 0:SBLK], in_=xT_r[:, k, 0:SBLK]
                )
                nc.sync.dma_start(
                    out=wqkv_sb[:, k, 2 * FSH : 3 * FSH],
                    in_=wqkv_r[:, k, 2 * FSH : 3 * FSH],
                )
            for k in range(KSUB):
                nc.sync.dma_start(
                    out=wqkv_sb[:, k, FSH : 2 * FSH],
                    in_=wqkv_r[:, k, FSH : 2 * FSH],
                )
            for p_ in range(NPAIR):
                nc.vector.memset(lhsT2_sb[p_], 0.0)
                nc.vector.memset(ksumrep_sb[p_], 0.0)

            # Non-critical DMAs queue behind the block-0 set on the same
            # hardware queue, so single-queue in-order dispatch gates them
            # without explicit dependencies.
            for j in range(1, NBLK):
                nc.sync.dma_start(
                    out=xt_sb[:, :, j * SBLK : (j + 1) * SBLK],
                    in_=xT_r[:, :, j * SBLK : (j + 1) * SBLK],
                )
            for k in range(KSUB):
                nc.sync.dma_start(
                    out=wqkv_sb[:, k, 0:FSH], in_=wqkv_r[:, k, 0:FSH]
                )
            nc.sync.dma_start(out=wo_sb, in_=wo_r)

            # SBUF pools shared across both phases
            etpool = ctx.enter_context(tc.tile_pool(name="et", bufs=3))
            qtpool = ctx.enter_context(tc.tile_pool(name="qt", bufs=2))
            qts = {}

            def qt_elu(ps, j, f):
                # elu(x)+1 = min(exp(x),1) + relu(x); Exp/Relu on ACT, the
                # combine on DVE
                e = etpool.tile([P, SBLK], F32, tag="e")
                nc.scalar.activation(out=e, in_=ps, func=ACT.Exp)
                r = etpool.tile([P, SBLK], F32, tag="r")
                nc.scalar.activation(out=r, in_=ps, func=ACT.Relu)
                nc.vector.scalar_tensor_tensor(
                    out=qts[j][:, f, :],
                    in0=e,
                    scalar=1.0,
                    in1=r,
                    op0=mybir.AluOpType.min,
                    op1=mybir.AluOpType.add,
                )

            # ---------------- phase 1: K,V projection + KV accumulation ----
            with (
                tc.tile_pool(name="kvps", bufs=1, space="PSUM") as kvps_pool,
                tc.tile_pool(name="pa", bufs=4, space="PSUM") as pa_pool,
                tc.tile_pool(name="st", bufs=2) as stpool,
            ):
                kvps = [
                    kvps_pool.tile([P, P + 1], F32, tag=f"kv{p}", name=f"kv{p}")
                    for p in range(NPAIR)
                ]

                bq = []  # lagged [KV | K_sum] accumulation entries

                def emit_b(ent):
                    kst, vst, j, t = ent
                    first = j == 0 and t == 0
                    last = j == NBLK - 1 and t == TSUB - 1
                    for p_ in range(NPAIR):
                        nc.tensor.matmul(
                            kvps[p_],
                            kst[:, t, p_ * P : (p_ + 1) * P],
                            vst[:, t, p_, :],
                            start=first,
                            stop=last,
                        )

                def elu_k(ps, kst, t):
                    e = etpool.tile([P, SBLK], F32, tag="e")
                    nc.scalar.activation(out=e, in_=ps, func=ACT.Exp)
                    r = etpool.tile([P, SBLK], F32, tag="r")
                    nc.vector.tensor_scalar_max(r, ps, 0.0)
                    nc.vector.scalar_tensor_tensor(
                        out=kst[:, t, :],
                        in0=e,
                        scalar=1.0,
                        in1=r,
                        op0=mybir.AluOpType.min,
                        op1=mybir.AluOpType.add,
                    )

                # block 0: V-sweep k-outer so PE work tracks DMA chunk
                # arrival (V needs no activation, so the K sweep that
                # follows runs at PE speed with elu pipelined per subtile)
                kst0 = stpool.tile([P, TSUB, FSH], BF16, tag="kst")
                vst0 = stpool.tile([P, TSUB, NPAIR, P + 1], BF16, tag="vst")
                nc.vector.memset(vst0[:, :, :, P : P + 1], 1.0)
                psvs = [
                    pa_pool.tile([P, SBLK], F32, tag="pa", name=f"psv{t}")
                    for t in range(TSUB)
                ]
                for k in range(KSUB):
                    for t in range(TSUB):
                        nc.tensor.matmul(
                            psvs[t],
                            xt_sb[:, k, t * P : (t + 1) * P],
                            wqkv_sb[:, k, 2 * FSH : 3 * FSH],
                            start=(k == 0),
                            stop=(k == KSUB - 1),
                        )
                for t in range(TSUB):
                    nc.scalar.copy(out=vst0[:, t, :, 0:P], in_=psvs[t])
                for t in range(TSUB):
                    psk = pa_pool.tile([P, SBLK], F32, tag="pa", name=f"psk{t}")
                    for k in range(KSUB):
                        nc.tensor.matmul(
                            psk,
                            xt_sb[:, k, t * P : (t + 1) * P],
                            wqkv_sb[:, k, FSH : 2 * FSH],
                            start=(k == 0),
                            stop=(k == KSUB - 1),
                        )
                    if t >= 1:
                        emit_b(bq.pop(0))
                    elu_k(psk, kst0, t)
                    bq.append((kst0, vst0, 0, t))

                # blocks 1..7: token-subtile-outer, B lagged one step
                for j in range(1, NBLK):
                    kst = stpool.tile([P, TSUB, FSH], BF16, tag="kst")
                    vst = stpool.tile([P, TSUB, NPAIR, P + 1], BF16, tag="vst")
                    nc.vector.memset(vst[:, :, :, P : P + 1], 1.0)
                    for t in range(TSUB):
                        tok = j * SBLK + t * P
                        psk = pa_pool.tile([P, SBLK], F32, tag="pa")
                        psv = pa_pool.tile([P, SBLK], F32, tag="pa")
                        for k in range(KSUB):
                            nc.tensor.matmul(
                                psk,
                                xt_sb[:, k, tok : tok + P],
                                wqkv_sb[:, k, FSH : 2 * FSH],
                                start=(k == 0),
                                stop=(k == KSUB - 1),
                            )
                            nc.tensor.matmul(
                                psv,
                                xt_sb[:, k, tok : tok + P],
                                wqkv_sb[:, k, 2 * FSH : 3 * FSH],
                                start=(k == 0),
                                stop=(k == KSUB - 1),
                            )
                        emit_b(bq.pop(0))
                        elu_k(psk, kst, t)
                        nc.scalar.copy(out=vst[:, t, :, 0:P], in_=psv)
                        bq.append((kst, vst, j, t))
                # block 0's Q projection runs here, inside the phase-1 PSUM
                # pools: it has no dependency on the KV state, so it keeps
                # the PE busy across the phase boundary (the trailing elu,
                # the KV extraction, and the phase-2 pool handover all hide
                # under its 32 matmuls)
                qts[0] = qtpool.tile([P, NPAIR, SBLK], BF16, tag="qt", name="qt0")
                for f in range(FSH // P):
                    psq = pa_pool.tile([P, SBLK], F32, tag="pa")
                    for k in range(KSUB):
                        nc.tensor.matmul(
                            psq,
                            wqkv_sb[:, k, f * P : (f + 1) * P],
                            xt_sb[:, k, 0:SBLK],
                            start=(k == 0),
                            stop=(k == KSUB - 1),
                        )
                    if f == 0:
                        emit_b(bq.pop(0))
                    qt_elu(psq, 0, f)

                # extraction: blockdiag KV + replicated K_sum (zeros preset)
                for p_ in range(NPAIR):
                    nc.vector.tensor_copy(
                        out=lhsT2_sb[p_][0:HD, 0:HD], in_=kvps[p_][0:HD, 0:HD]
                    )
                    nc.vector.tensor_copy(
                        out=lhsT2_sb[p_][HD:P, HD:P], in_=kvps[p_][HD:P, HD:P]
                    )
                    nc.vector.tensor_copy(
                        out=ksumrep_sb[p_][0:HD, 0:HD],
                        in_=kvps[p_][0:HD, P : P + 1].to_broadcast((HD, HD)),
                    )
                    nc.vector.tensor_copy(
                        out=ksumrep_sb[p_][HD:P, HD:P],
                        in_=kvps[p_][HD:P, P : P + 1].to_broadcast((HD, HD)),
                    )

            # ---------------- phase 2: Q projection + attention + Wo -------
            with (
                tc.tile_pool(name="mm512", bufs=3, space="PSUM") as mmps,
                tc.tile_pool(name="pc", bufs=3, space="PSUM") as pcps,
                tc.tile_pool(name="pnb", bufs=2, space="PSUM") as pnps,
                tc.tile_pool(name="ou", bufs=3) as oupool,
                tc.tile_pool(name="rc", bufs=4) as rcpool,
                tc.tile_pool(name="ot", bufs=2) as otpool,
                tc.tile_pool(name="ys", bufs=4) as ypool,
            ):
                outus = {}
                rcbs = {}
                outts = {}

                def qt_half(j, fh):
                    if j not in qts:
                        qts[j] = qtpool.tile(
                            [P, NPAIR, SBLK], BF16, tag="qt", name=f"qt{j}"
                        )
                    for f in (2 * fh, 2 * fh + 1):
                        ps = mmps.tile([P, SBLK], F32, tag="mm")
                        for k in range(KSUB):
                            nc.tensor.matmul(
                                ps,
                                wqkv_sb[:, k, f * P : (f + 1) * P],
                                xt_sb[:, k, j * SBLK : (j + 1) * SBLK],
                                start=(k == 0),
                                stop=(k == KSUB - 1),
                            )
                        qt_elu(ps, j, f)

                def attn_pairs(j, pairs):
                    # per pair: attention matmul + broadcast-normalizer
                    # matmul; the DVE apply-multiply reads psc directly from
                    # PSUM (no ACT eviction, one less latency link)
                    qtj = qts[j]
                    if j not in outts:
                        outts[j] = otpool.tile(
                            [P, NPAIR, SBLK], BF16, tag="outt", name="outt"
                        )
                    outt = outts[j]
                    for p_ in pairs:
                        psc = pcps.tile([P, SBLK], F32, tag="pc", name="psc")
                        nc.tensor.matmul(
                            psc,
                            lhsT2_sb[p_],
                            qtj[:, p_, :],
                            start=True,
                            stop=True,
                        )
                        psn = pnps.tile([P, SBLK], F32, tag="pn", name="psn")
                        nc.tensor.matmul(
                            psn,
                            ksumrep_sb[p_],
                            qtj[:, p_, :],
                            start=True,
                            stop=True,
                        )
                        rcb = rcpool.tile([P, SBLK], F32, tag="rcb", name="rcb")
                        nc.vector.reciprocal_approx_fast(out=rcb[:], in_=psn[:])
                        nc.vector.tensor_tensor(
                            out=outt[:, p_, :],
                            in0=psc[:],
                            in1=rcb[:],
                            op=mybir.AluOpType.mult,
                        )

                def psc_section(j):
                    # finale-only variant: ACT-evicts psc to outu so the
                    # apply can be split per token subtile in the drain
                    qtj = qts.pop(j)
                    outu = oupool.tile([P, NPAIR, SBLK], F32, tag="outu")
                    outus[j] = outu
                    rcbs[j] = []
                    for p_ in range(NPAIR):
                        psc = pcps.tile([P, SBLK], F32, tag="pc")
                        nc.tensor.matmul(
                            psc,
                            lhsT2_sb[p_],
                            qtj[:, p_, :],
                            start=True,
                            stop=True,
                        )
                        nc.scalar.copy(out=outu[:, p_, :], in_=psc)
                        psn = pnps.tile([P, SBLK], F32, tag="pn")
                        nc.tensor.matmul(
                            psn,
                            ksumrep_sb[p_],
                            qtj[:, p_, :],
                            start=True,
                            stop=True,
                        )
                        rcb = rcpool.tile([P, SBLK], F32, tag="rcb")
                        nc.vector.reciprocal_approx_fast(out=rcb[:], in_=psn[:])
                        rcbs[j].append(rcb)

                def d_t(j, outt, t, drain=False):
                    ysb = ypool.tile([P, D], F32, tag="ysb", name="ysb")
                    psy0 = mmps.tile([P, 512], F32, tag="mm", name="psy0")
                    psy1 = mmps.tile([P, 512], F32, tag="mm", name="psy1")
                    for fs in range(FSH // P):
                        nc.tensor.matmul(
                            psy0,
                            outt[:, fs, t * P : (t + 1) * P],
                            wo_sb[:, fs, 0:512],
                            start=(fs == 0),
                            stop=(fs == FSH // P - 1),
                        )
                        nc.tensor.matmul(
                            psy1,
                            outt[:, fs, t * P : (t + 1) * P],
                            wo_sb[:, fs, 512:1024],
                            start=(fs == 0),
                            stop=(fs == FSH // P - 1),
                        )
                    nc.scalar.copy(out=ysb[:, 0:512], in_=psy0)
                    nc.sync.dma_start(out=y_rt[j, t, 0], in_=ysb[:, 0:512])
                    if drain:
                        # DVE is otherwise idle in the drain; parallel evict
                        nc.vector.tensor_copy(out=ysb[:, 512:1024], in_=psy1)
                    else:
                        nc.scalar.copy(out=ysb[:, 512:1024], in_=psy1)
                    nc.sync.dma_start(out=y_rt[j, t, 1], in_=ysb[:, 512:1024])

                def d_block(j):
                    outt = outts.pop(j)
                    for t in range(TSUB):
                        d_t(j, outt, t)

                def finale(j):
                    # drain block: apply-multiplies split per token subtile
                    # so each D chain starts as soon as its slice is scaled
                    outt = otpool.tile([P, NPAIR, SBLK], BF16, tag="outt")
                    outu = outus.pop(j)
                    rcs = rcbs.pop(j)
                    for t in range(TSUB):
                        sl = slice(t * P, (t + 1) * P)
                        for p_ in range(NPAIR):
                            nc.vector.tensor_tensor(
                                out=outt[:, p_, sl],
                                in0=outu[:, p_, sl],
                                in1=rcs[p_][:, sl],
                                op=mybir.AluOpType.mult,
                            )
                        d_t(j, outt, t, drain=(t == TSUB - 1))

                # steady-state emission: block j's Q projection brackets
                # block j-1's attention chain so the PE never waits on the
                # ACT/DVE eviction+reciprocal+apply latency.
                for j in range(1, NBLK):
                    attn_pairs(j - 1, [0, 1])
                    attn_pairs(j - 1, [2, 3])
                    qt_half(j, 0)
                    qt_half(j, 1)
                    if j == NBLK - 1:
                        # last block's attention section runs ahead of
                        # d_block(6) so its evictions and reciprocals hide
                        # under D's matmuls and the drain starts immediately
                        psc_section(NBLK - 1)
                    d_block(j - 1)
                finale(NBLK - 1)

    nc.compile()
    return nc


def _prep_inputs(x, Wqkv, Wo):
    import ml_dtypes

    x = np.ascontiguousarray(x, dtype=np.float32)
    Wqkv = np.ascontiguousarray(Wqkv, dtype=np.float32)
    Wo = np.ascontiguousarray(Wo, dtype=np.float32)
    in_maps = []
    for b in range(B):
        xT = np.ascontiguousarray(x[b].T).astype(ml_dtypes.bfloat16)  # [D, S]
        for hh in range(2):
            cols = slice(hh * FSH, (hh + 1) * FSH)
            wq = Wqkv[:, 0 * D :][:, cols]
            wk = Wqkv[:, 1 * D :][:, cols]
            wv = Wqkv[:, 2 * D :][:, cols]
            wqkv_sh = np.ascontiguousarray(
                np.concatenate([wq, wk, wv], axis=1)
            ).astype(ml_dtypes.bfloat16)
            wo_sh = np.ascontiguousarray(Wo[hh * FSH : (hh + 1) * FSH, :]).astype(
                ml_dtypes.bfloat16
            )
            in_maps.append({"xT": xT, "wqkv": wqkv_sh, "wo": wo_sh})
    return in_maps


def kernel(x, Wqkv, Wo):
    global _NC_CACHE
    if _NC_CACHE is None:
        _NC_CACHE = build()
    nc = _NC_CACHE
    in_maps = _prep_inputs(x, Wqkv, Wo)
    res = run_bass_kernel_spmd(nc, in_maps, list(range(2 * B))).results
    y = np.empty((B, S, D), dtype=np.float32)
    for b in range(B):
        y[b] = res[2 * b]["y"] + res[2 * b + 1]["y"]
    return y



# revision 2
# speedup vs baseline: 1.1304x; 1.1304x over previous
"""Linear attention (B=4, S=4096, D=1024, H=16) on 8 TRN2 NeuronCores.

Sharding: core = (batch, head-half): each core handles one batch's 8 heads.
 - x is host-transposed to xT [D, S] per batch so both operand orientations
   of every matmul come out of the tensor engine with no on-device transpose.
 - Wqkv column-sharded per head-half; Wo row-sharded; host sums the two
   partial y's per batch (row-parallel unshard).

v2: K and Q projections run in fp8e4 DoubleRow (2 contraction rows per PE
cell, ~1.8x per-matmul throughput). Errors in K/Q largely cancel through
the attention normalizer (measured end-to-end ~1.25e-2 vs 2e-2 budget);
V/out-proj paths stay bf16 (their fp8 error flows straight to the output).
Host ships x8 = e4m3(xT*16) and wqk8 = e4m3([Wq|Wk]*512); the 1/8192
descale folds into the ACT activation scale of the elu evaluation, and the
K-path relu becomes a DVE scalar_tensor_tensor multiply with a constant
tile, so ACT/DVE load is unchanged vs the bf16 kernel.

Two-phase dataflow (V/out matmuls bf16, fp32 PSUM accumulate):

phase 1 (per 512-token block): K projection fp8-DR (4 k-pair matmuls per
  128-token subtile) -> elu+1(K); V projection bf16 (8 k matmuls) ->
  [KV | K_sum^T] PSUM accumulation per head-pair (vst carries a ones
  column so one matmul does both). bf16 x is streamed per block (4-deep
  pool) since only the V projection reads it; x8 stays fully resident.
  Block 0 runs the K projection k-pair-outer (4 simultaneous PSUM chains)
  so compute starts as soon as the first (x8, wk8) DMA chunk lands.
  Block 0's Q projection runs at the end of phase 1 to bridge the
  phase transition with PE work that has no KV dependency.

phase 2 (per block, software-pipelined across j):
  QT [512f, 512s] feature-major fp8-DR (lhsT=wq8 pair, rhs=x8 pair) ->
  elu+1 -> bf16
  psc[128,s] = blockdiag(KV_h0, KV_h1)^T @ QT_pair (bf16); norm via the
  replicated-ksum matmul; 1/x on DVE fast reciprocal; outT = outu * rcp
  y[s,:] = outT^T @ Wo per 128-token subtile, fp32, one 512KB DMA per
  subtile.
"""

import numpy as np

import concourse.bacc as bacc
import concourse.mybir as mybir
import concourse.tile as tile
from concourse.bass_utils import run_bass_kernel_spmd

F32 = mybir.dt.float32
BF16 = mybir.dt.bfloat16
F8 = mybir.dt.float8e4
ACT = mybir.ActivationFunctionType
DR = mybir.MatmulPerfMode.DoubleRow

P = 128
B, S, D = 4, 4096, 1024
H = 16
HD = 64

FSH = 512            # features per core for each of Q, K, V (8 heads)
KSUB = D // P        # 8 contraction subtiles
KPAIR = KSUB // 2    # 4 fp8 DoubleRow contraction pairs
SBLK = 512           # tokens per block
NBLK = S // SBLK     # 8 blocks
TSUB = SBLK // P     # 4 token subtiles per block
NPAIR = 4            # head pairs per core

SX = 16.0            # fp8 pre-scale on x
SW = 512.0           # fp8 pre-scale on Wq/Wk
INV = 1.0 / (SX * SW)

_NC_CACHE = None


def build():
    nc = bacc.Bacc(target_bir_lowering=False)
    x8 = nc.dram_tensor("x8", [D, S], F8, kind="ExternalInput")
    xT = nc.dram_tensor("xT", [D, S], BF16, kind="ExternalInput")
    wqk8 = nc.dram_tensor("wqk8", [D, 2 * FSH], F8, kind="ExternalInput")
    wv = nc.dram_tensor("wv", [D, FSH], BF16, kind="ExternalInput")
    wo = nc.dram_tensor("wo", [FSH, D], BF16, kind="ExternalInput")
    y = nc.dram_tensor("y", [S, D], F32, kind="ExternalOutput")

    x8_r = x8.rearrange("(ko p) s -> p ko s", p=P)        # [128, 8, 4096]
    xT_r = xT.rearrange("(ko p) s -> p ko s", p=P)        # [128, 8, 4096]
    wqk8_r = wqk8.rearrange("(ko p) f -> p ko f", p=P)    # [128, 8, 1024]
    wv_r = wv.rearrange("(ko p) f -> p ko f", p=P)        # [128, 8, 512]
    wo_r = wo.rearrange("(fo p) n -> p fo n", p=P)        # [128, 4, 1024]
    y_rt = y.rearrange(
        "(j t p) (nh n) -> j t nh p n", t=TSUB, p=P, nh=2
    )  # [8,4,2,128,512]
    y_rb = y.rearrange("(j t p) d -> j t p d", t=TSUB, p=P)  # [8,4,128,1024]

    with tile.TileContext(nc) as tc:
        import contextlib

        with contextlib.ExitStack() as ctx:
            wpool = ctx.enter_context(tc.tile_pool(name="wpool", bufs=1))

            # persistent SBUF
            x8_sb = wpool.tile([P, KSUB, S], F8)            # all of x8, 32KB/p
            wqk8_sb = wpool.tile([P, KSUB, 2 * FSH], F8)    # [wq8|wk8]
            wv_sb = wpool.tile([P, KSUB, FSH], BF16)
            wo_sb = wpool.tile([P, FSH // P, D], BF16)
            cinv = wpool.tile([P, SBLK], F32)               # INV const tile
            # per-pair block-diagonal [[KV_h0, 0], [0, KV_h1]] (128x128)
            lhsT2_sb = [
                wpool.tile([P, P], BF16, name=f"l2{p}") for p in range(NPAIR)
            ]
            # per-pair [ksum_h0 x64 | ksum_h1 x64] replicated along free dim
            ksumrep_sb = [
                wpool.tile([P, P], BF16, name=f"kr{p}") for p in range(NPAIR)
            ]

            xbfpool = ctx.enter_context(tc.tile_pool(name="xbf", bufs=4))
            xbfs = []

            # Startup-critical transfers first, interleaved per k-pair so the
            # block-0 K projection starts on the first chunks; then the rest
            # in a handful of large transfers. All in-order on the sync
            # queue, so later DMAs are naturally gated behind earlier ones.
            for i in range(KPAIR):
                nc.sync.dma_start(
                    out=x8_sb[:, 2 * i : 2 * i + 2, 0:SBLK],
                    in_=x8_r[:, 2 * i : 2 * i + 2, 0:SBLK],
                )
                nc.sync.dma_start(
                    out=wqk8_sb[:, 2 * i : 2 * i + 2, FSH : 2 * FSH],
                    in_=wqk8_r[:, 2 * i : 2 * i + 2, FSH : 2 * FSH],
                )
            nc.sync.dma_start(out=wv_sb, in_=wv_r)
            xbf0 = xbfpool.tile([P, KSUB, SBLK], BF16, tag="xbf", name="xbf0")
            xbfs.append(xbf0)
            nc.sync.dma_start(out=xbf0, in_=xT_r[:, :, 0:SBLK])
            nc.sync.dma_start(
                out=x8_sb[:, :, SBLK : 4 * SBLK], in_=x8_r[:, :, SBLK : 4 * SBLK]
            )
            nc.sync.dma_start(
                out=x8_sb[:, :, 4 * SBLK :], in_=x8_r[:, :, 4 * SBLK :]
            )
            nc.sync.dma_start(
                out=wqk8_sb[:, :, 0:FSH], in_=wqk8_r[:, :, 0:FSH]
            )
            nc.sync.dma_start(out=wo_sb, in_=wo_r)
            for j in range(1, NBLK):
                xb = xbfpool.tile([P, KSUB, SBLK], BF16, tag="xbf", name=f"xbf{j}")
                xbfs.append(xb)
                nc.sync.dma_start(
                    out=xb, in_=xT_r[:, :, j * SBLK : (j + 1) * SBLK]
                )

            nc.vector.memset(cinv, INV)
            for p_ in range(NPAIR):
                nc.vector.memset(lhsT2_sb[p_], 0.0)
                nc.vector.memset(ksumrep_sb[p_], 0.0)

            # SBUF pools shared across both phases
            etpool = ctx.enter_context(tc.tile_pool(name="et", bufs=3))
            qtpool = ctx.enter_context(tc.tile_pool(name="qt", bufs=2))
            qts = {}

            def qt_elu(ps, j, f):
                # elu(z)+1 = min(exp(z),1) + relu(z), z = ps*INV (fp8
                # descale); Exp/Relu on ACT with the scale pre-op, the
                # combine on DVE
                e = etpool.tile([P, SBLK], F32, tag="e")
                nc.scalar.activation(out=e, in_=ps, func=ACT.Exp, scale=INV)
                r = etpool.tile([P, SBLK], F32, tag="r")
                nc.scalar.activation(out=r, in_=ps, func=ACT.Relu, scale=INV)
                nc.vector.scalar_tensor_tensor(
                    out=qts[j][:, f, :],
                    in0=e,
                    scalar=1.0,
                    in1=r,
                    op0=mybir.AluOpType.min,
                    op1=mybir.AluOpType.add,
                )

            def dr_mm(ps, lhsT, rhs, i):
                nc.tensor.matmul(
                    ps,
                    lhsT,
                    rhs,
                    start=(i == 0),
                    stop=(i == KPAIR - 1),
                    perf_mode=DR,
                )

            # ---------------- phase 1: K,V projection + KV accumulation ----
            with (
                tc.tile_pool(name="kvps", bufs=1, space="PSUM") as kvps_pool,
                tc.tile_pool(name="pa", bufs=4, space="PSUM") as pa_pool,
                tc.tile_pool(name="st", bufs=2) as stpool,
            ):
                kvps = [
                    kvps_pool.tile([P, P + 1], F32, tag=f"kv{p}", name=f"kv{p}")
                    for p in range(NPAIR)
                ]

                bq = []  # lagged [KV | K_sum] accumulation entries

                def emit_b(ent):
                    kst, vst, j, t = ent
                    first = j == 0 and t == 0
                    last = j == NBLK - 1 and t == TSUB - 1
                    for p_ in range(NPAIR):
                        nc.tensor.matmul(
                            kvps[p_],
                            kst[:, t, p_ * P : (p_ + 1) * P],
                            vst[:, t, p_, :],
                            start=first,
                            stop=last,
                        )

                def elu_k(ps, kst, t):
                    # e = exp(ps*INV) on ACT; r = max(ps,0)*INV on DVE via
                    # the cinv const tile; combine min(e,1)+r on DVE
                    e = etpool.tile([P, SBLK], F32, tag="e")
                    nc.scalar.activation(out=e, in_=ps, func=ACT.Exp, scale=INV)
                    r = etpool.tile([P, SBLK], F32, tag="r")
                    nc.vector.scalar_tensor_tensor(
                        out=r,
                        in0=ps,
                        scalar=0.0,
                        in1=cinv,
                        op0=mybir.AluOpType.max,
                        op1=mybir.AluOpType.mult,
                    )
                    nc.vector.scalar_tensor_tensor(
                        out=kst[:, t, :],
                        in0=e,
                        scalar=1.0,
                        in1=r,
                        op0=mybir.AluOpType.min,
                        op1=mybir.AluOpType.add,
                    )

                # block 0: K-sweep k-pair-outer so PE work tracks DMA chunk
                # arrival (4 simultaneous PSUM chains, one per 128-token
                # subtile); V follows t-outer once wv/xbf0 have landed
                kst0 = stpool.tile([P, TSUB, FSH], BF16, tag="kst")
                vst0 = stpool.tile([P, TSUB, NPAIR, P + 1], BF16, tag="vst")
                nc.vector.memset(vst0[:, :, :, P : P + 1], 1.0)
                psks = [
                    pa_pool.tile([P, SBLK], F32, tag="pa", name=f"psk{t}")
                    for t in range(TSUB)
                ]
                for i in range(KPAIR):
                    for t in range(TSUB):
                        dr_mm(
                            psks[t],
                            x8_sb[:, 2 * i : 2 * i + 2, t * P : (t + 1) * P],
                            wqk8_sb[:, 2 * i : 2 * i + 2, FSH : 2 * FSH],
                            i,
                        )
                for t in range(TSUB):
                    elu_k(psks[t], kst0, t)
                for t in range(TSUB):
                    psv = pa_pool.tile([P, SBLK], F32, tag="pa", name=f"psv{t}")
                    for k in range(KSUB):
                        nc.tensor.matmul(
                            psv,
                            xbf0[:, k, t * P : (t + 1) * P],
                            wv_sb[:, k, :],
                            start=(k == 0),
                            stop=(k == KSUB - 1),
                        )
                    if t >= 1:
                        emit_b(bq.pop(0))
                    nc.scalar.copy(out=vst0[:, t, :, 0:P], in_=psv)
                    bq.append((kst0, vst0, 0, t))

                # blocks 1..7: token-subtile-outer, B lagged one step
                for j in range(1, NBLK):
                    kst = stpool.tile([P, TSUB, FSH], BF16, tag="kst")
                    vst = stpool.tile([P, TSUB, NPAIR, P + 1], BF16, tag="vst")
                    nc.vector.memset(vst[:, :, :, P : P + 1], 1.0)
                    xbf = xbfs[j]
                    for t in range(TSUB):
                        tok = j * SBLK + t * P
                        psk = pa_pool.tile([P, SBLK], F32, tag="pa")
                        psv = pa_pool.tile([P, SBLK], F32, tag="pa")
                        for i in range(KPAIR):
                            dr_mm(
                                psk,
                                x8_sb[:, 2 * i : 2 * i + 2, tok : tok + P],
                                wqk8_sb[:, 2 * i : 2 * i + 2, FSH : 2 * FSH],
                                i,
                            )
                        for k in range(KSUB):
                            nc.tensor.matmul(
                                psv,
                                xbf[:, k, t * P : (t + 1) * P],
                                wv_sb[:, k, :],
                                start=(k == 0),
                                stop=(k == KSUB - 1),
                            )
                        emit_b(bq.pop(0))
                        elu_k(psk, kst, t)
                        nc.scalar.copy(out=vst[:, t, :, 0:P], in_=psv)
                        bq.append((kst, vst, j, t))
                # block 0's Q projection runs here, inside the phase-1 PSUM
                # pools: it has no dependency on the KV state, so it keeps
                # the PE busy across the phase boundary
                qts[0] = qtpool.tile([P, NPAIR, SBLK], BF16, tag="qt", name="qt0")
                for f in range(FSH // P):
                    psq = pa_pool.tile([P, SBLK], F32, tag="pa")
                    for i in range(KPAIR):
                        dr_mm(
                            psq,
                            wqk8_sb[:, 2 * i : 2 * i + 2, f * P : (f + 1) * P],
                            x8_sb[:, 2 * i : 2 * i + 2, 0:SBLK],
                            i,
                        )
                    if f == 0:
                        emit_b(bq.pop(0))
                    qt_elu(psq, 0, f)

                # extraction: blockdiag KV + replicated K_sum (zeros preset)
                for p_ in range(NPAIR):
                    nc.vector.tensor_copy(
                        out=lhsT2_sb[p_][0:HD, 0:HD], in_=kvps[p_][0:HD, 0:HD]
                    )
                    nc.vector.tensor_copy(
                        out=lhsT2_sb[p_][HD:P, HD:P], in_=kvps[p_][HD:P, HD:P]
                    )
                    nc.vector.tensor_copy(
                        out=ksumrep_sb[p_][0:HD, 0:HD],
                        in_=kvps[p_][0:HD, P : P + 1].to_broadcast((HD, HD)),
                    )
                    nc.vector.tensor_copy(
                        out=ksumrep_sb[p_][HD:P, HD:P],
                        in_=kvps[p_][HD:P, P : P + 1].to_broadcast((HD, HD)),
                    )

            # ---------------- phase 2: Q projection + attention + Wo -------
            with (
                tc.tile_pool(name="mm512", bufs=3, space="PSUM") as mmps,
                tc.tile_pool(name="pc", bufs=3, space="PSUM") as pcps,
                tc.tile_pool(name="pnb", bufs=2, space="PSUM") as pnps,
                tc.tile_pool(name="ou", bufs=3) as oupool,
                tc.tile_pool(name="rc", bufs=4) as rcpool,
                tc.tile_pool(name="ot", bufs=2) as otpool,
                tc.tile_pool(name="ys", bufs=4) as ypool,
            ):
                outus = {}
                rcbs = {}
                outts = {}

                def qt_half(j, fh):
                    if j not in qts:
                        qts[j] = qtpool.tile(
                            [P, NPAIR, SBLK], BF16, tag="qt", name=f"qt{j}"
                        )
                    for f in (2 * fh, 2 * fh + 1):
                        ps = mmps.tile([P, SBLK], F32, tag="mm")
                        for i in range(KPAIR):
                            dr_mm(
                                ps,
                                wqk8_sb[:, 2 * i : 2 * i + 2, f * P : (f + 1) * P],
                                x8_sb[:, 2 * i : 2 * i + 2, j * SBLK : (j + 1) * SBLK],
                                i,
                            )
                        qt_elu(ps, j, f)

                def attn_pairs(j, pairs):
                    # per pair: attention matmul + broadcast-normalizer
                    # matmul; the DVE apply-multiply reads psc directly from
                    # PSUM (no ACT eviction, one less latency link)
                    qtj = qts[j]
                    if j not in outts:
                        outts[j] = otpool.tile(
                            [P, NPAIR, SBLK], BF16, tag="outt", name="outt"
                        )
                    outt = outts[j]
                    for p_ in pairs:
                        psc = pcps.tile([P, SBLK], F32, tag="pc", name="psc")
                        nc.tensor.matmul(
                            psc,
                            lhsT2_sb[p_],
                            qtj[:, p_, :],
                            start=True,
                            stop=True,
                        )
                        psn = pnps.tile([P, SBLK], F32, tag="pn", name="psn")
                        nc.tensor.matmul(
                            psn,
                            ksumrep_sb[p_],
                            qtj[:, p_, :],
                            start=True,
                            stop=True,
                        )
                        rcb = rcpool.tile([P, SBLK], F32, tag="rcb", name="rcb")
                        nc.vector.reciprocal_approx_fast(out=rcb[:], in_=psn[:])
                        nc.vector.tensor_tensor(
                            out=outt[:, p_, :],
                            in0=psc[:],
                            in1=rcb[:],
                            op=mybir.AluOpType.mult,
                        )

                def psc_section(j):
                    # finale-only variant: ACT-evicts psc to outu so the
                    # apply can be split per token subtile in the drain
                    qtj = qts.pop(j)
                    outu = oupool.tile([P, NPAIR, SBLK], F32, tag="outu")
                    outus[j] = outu
                    rcbs[j] = []
                    for p_ in range(NPAIR):
                        psc = pcps.tile([P, SBLK], F32, tag="pc")
                        nc.tensor.matmul(
                            psc,
                            lhsT2_sb[p_],
                            qtj[:, p_, :],
                            start=True,
                            stop=True,
                        )
                        nc.scalar.copy(out=outu[:, p_, :], in_=psc)
                        psn = pnps.tile([P, SBLK], F32, tag="pn")
                        nc.tensor.matmul(
                            psn,
                            ksumrep_sb[p_],
                            qtj[:, p_, :],
                            start=True,
                            stop=True,
                        )
                        rcb = rcpool.tile([P, SBLK], F32, tag="rcb")
                        nc.vector.reciprocal_approx_fast(out=rcb[:], in_=psn[:])
                        rcbs[j].append(rcb)

                def d_t(j, outt, t, drain=False):
                    ysb = ypool.tile([P, D], F32, tag="ysb", name="ysb")
                    psy0 = mmps.tile([P, 512], F32, tag="mm", name="psy0")
                    psy1 = mmps.tile([P, 512], F32, tag="mm", name="psy1")
                    for fs in range(FSH // P):
                        nc.tensor.matmul(
                            psy0,
                            outt[:, fs, t * P : (t + 1) * P],
                            wo_sb[:, fs, 0:512],
                            start=(fs == 0),
                            stop=(fs == FSH // P - 1),
                        )
                        nc.tensor.matmul(
                            psy1,
                            outt[:, fs, t * P : (t + 1) * P],
                            wo_sb[:, fs, 512:1024],
                            start=(fs == 0),
                            stop=(fs == FSH // P - 1),
                        )
                    if drain:
                        # DVE is otherwise idle in the drain; parallel evict
                        # and split the DMA per half so it starts earlier
                        nc.scalar.copy(out=ysb[:, 0:512], in_=psy0)
                        nc.sync.dma_start(out=y_rt[j, t, 0], in_=ysb[:, 0:512])
                        nc.vector.tensor_copy(out=ysb[:, 512:1024], in_=psy1)
                        nc.sync.dma_start(out=y_rt[j, t, 1], in_=ysb[:, 512:1024])
                    else:
                        nc.scalar.copy(out=ysb[:, 0:512], in_=psy0)
                        nc.scalar.copy(out=ysb[:, 512:1024], in_=psy1)
                        nc.sync.dma_start(out=y_rb[j, t], in_=ysb)

                def d_block(j):
                    outt = outts.pop(j)
                    for t in range(TSUB):
                        d_t(j, outt, t)

                def finale(j):
                    # drain block: apply-multiplies split per token subtile
                    # so each D chain starts as soon as its slice is scaled
                    outt = otpool.tile([P, NPAIR, SBLK], BF16, tag="outt")
                    outu = outus.pop(j)
                    rcs = rcbs.pop(j)
                    for t in range(TSUB):
                        sl = slice(t * P, (t + 1) * P)
                        for p_ in range(NPAIR):
                            nc.vector.tensor_tensor(
                                out=outt[:, p_, sl],
                                in0=outu[:, p_, sl],
                                in1=rcs[p_][:, sl],
                                op=mybir.AluOpType.mult,
                            )
                        d_t(j, outt, t, drain=(t == TSUB - 1))

                # steady-state emission: block j's Q projection brackets
                # block j-1's attention chain so the PE never waits on the
                # ACT/DVE eviction+reciprocal+apply latency.
                for j in range(1, NBLK):
                    attn_pairs(j - 1, [0, 1])
                    attn_pairs(j - 1, [2, 3])
                    qt_half(j, 0)
                    qt_half(j, 1)
                    if j == NBLK - 1:
                        # last block's attention section runs ahead of
                        # d_block(6) so its evictions and reciprocals hide
                        # under D's matmuls and the drain starts immediately
                        psc_section(NBLK - 1)
                    d_block(j - 1)
                finale(NBLK - 1)

    nc.compile()
    return nc


def _prep_inputs(x, Wqkv, Wo):
    import ml_dtypes

    x = np.ascontiguousarray(x, dtype=np.float32)
    Wqkv = np.ascontiguousarray(Wqkv, dtype=np.float32)
    Wo = np.ascontiguousarray(Wo, dtype=np.float32)

    def f8(a):
        return np.clip(a, -240.0, 240.0).astype(ml_dtypes.float8_e4m3fn)

    in_maps = []
    for b in range(B):
        xTb = np.ascontiguousarray(x[b].T)                 # [D, S] fp32
        xT = xTb.astype(ml_dtypes.bfloat16)
        x8 = f8(xTb * SX)
        for hh in range(2):
            cols = slice(hh * FSH, (hh + 1) * FSH)
            wq = Wqkv[:, 0 * D :][:, cols]
            wk = Wqkv[:, 1 * D :][:, cols]
            wv = Wqkv[:, 2 * D :][:, cols]
            wqk8 = f8(
                np.ascontiguousarray(np.concatenate([wq, wk], axis=1)) * SW
            )
            wv_sh = np.ascontiguousarray(wv).astype(ml_dtypes.bfloat16)
            wo_sh = np.ascontiguousarray(Wo[hh * FSH : (hh + 1) * FSH, :]).astype(
                ml_dtypes.bfloat16
            )
            in_maps.append(
                {"xT": xT, "x8": x8, "wqk8": wqk8, "wv": wv_sh, "wo": wo_sh}
            )
    return in_maps


def kernel(x, Wqkv, Wo):
    global _NC_CACHE
    if _NC_CACHE is None:
        _NC_CACHE = build()
    nc = _NC_CACHE
    in_maps = _prep_inputs(x, Wqkv, Wo)
    res = run_bass_kernel_spmd(nc, in_maps, list(range(2 * B))).results
    y = np.empty((B, S, D), dtype=np.float32)
    for b in range(B):
        y[b] = res[2 * b]["y"] + res[2 * b + 1]["y"]
    return y


# revision 8
# speedup vs baseline: 1.1768x; 1.0411x over previous
"""Linear attention (B=4, S=4096, D=1024, H=16) on 8 TRN2 NeuronCores.

Sharding: core = (batch, head-half): each core handles one batch's 8 heads.
 - x is host-transposed to xT [D, S] per batch so both operand orientations
   of every matmul come out of the tensor engine with no on-device transpose.
 - Wqkv column-sharded per head-half; Wo row-sharded; host sums the two
   partial y's per batch (row-parallel unshard).

v2: K and Q projections run in fp8e4 DoubleRow (2 contraction rows per PE
cell, ~1.8x per-matmul throughput). Errors in K/Q largely cancel through
the attention normalizer (measured end-to-end ~1.25e-2 vs 2e-2 budget);
V/out-proj paths stay bf16 (their fp8 error flows straight to the output).
Host ships x8 = e4m3(xT*16) and wqk8 = e4m3([Wq|Wk]*512); the 1/8192
descale folds into the ACT activation scale of the elu evaluation, and the
K-path relu becomes a DVE scalar_tensor_tensor multiply with a constant
tile, so ACT/DVE load is unchanged vs the bf16 kernel.

Two-phase dataflow (V/out matmuls bf16, fp32 PSUM accumulate):

phase 1 (per 512-token block): K projection fp8-DR (4 k-pair matmuls per
  128-token subtile) -> elu+1(K); V projection bf16 (8 k matmuls) ->
  [KV | K_sum^T] PSUM accumulation per head-pair (vst carries a ones
  column so one matmul does both). bf16 x is streamed per block (4-deep
  pool) since only the V projection reads it; x8 stays fully resident.
  Block 0 runs the K projection k-pair-outer (4 simultaneous PSUM chains)
  so compute starts as soon as the first (x8, wk8) DMA chunk lands.
  Block 0's Q projection runs at the end of phase 1 to bridge the
  phase transition with PE work that has no KV dependency.

phase 2 (per block, software-pipelined across j):
  QT [512f, 512s] feature-major fp8-DR (lhsT=wq8 pair, rhs=x8 pair) ->
  elu+1 -> bf16
  psc[128,s] = blockdiag(KV_h0, KV_h1)^T @ QT_pair (bf16); norm via the
  replicated-ksum matmul; 1/x on DVE fast reciprocal; outT = outu * rcp
  y[s,:] = outT^T @ Wo per 128-token subtile, fp32, one 512KB DMA per
  subtile.
"""

import numpy as np

import concourse.bacc as bacc
import concourse.mybir as mybir
import concourse.tile as tile
from concourse.bass_utils import run_bass_kernel_spmd

F32 = mybir.dt.float32
BF16 = mybir.dt.bfloat16
F8 = mybir.dt.float8e4
ACT = mybir.ActivationFunctionType
DR = mybir.MatmulPerfMode.DoubleRow

P = 128
B, S, D = 4, 4096, 1024
H = 16
HD = 64

FSH = 512            # features per core for each of Q, K, V (8 heads)
KSUB = D // P        # 8 contraction subtiles
KPAIR = KSUB // 2    # 4 fp8 DoubleRow contraction pairs
SBLK = 512           # tokens per block
NBLK = S // SBLK     # 8 blocks
TSUB = SBLK // P     # 4 token subtiles per block
NPAIR = 4            # head pairs per core

SX = 16.0            # fp8 pre-scale on x
SW = 512.0           # fp8 pre-scale on Wq/Wk
INV = 1.0 / (SX * SW)

_NC_CACHE = None


def build():
    nc = bacc.Bacc(target_bir_lowering=False)
    xT = nc.dram_tensor("xT", [D, S], BF16, kind="ExternalInput")
    wqk8 = nc.dram_tensor("wqk8", [D, 2 * FSH], F8, kind="ExternalInput")
    wv = nc.dram_tensor("wv", [D, FSH], BF16, kind="ExternalInput")
    wo = nc.dram_tensor("wo", [FSH, D], BF16, kind="ExternalInput")
    y = nc.dram_tensor("y", [S, D], F32, kind="ExternalOutput")

    xT_r = xT.rearrange("(ko p) s -> p ko s", p=P)        # [128, 8, 4096]
    wqk8_r = wqk8.rearrange("(ko p) f -> p ko f", p=P)    # [128, 8, 1024]
    wv_r = wv.rearrange("(ko p) f -> p ko f", p=P)        # [128, 8, 512]
    wo_r = wo.rearrange("(fo p) n -> p fo n", p=P)        # [128, 4, 1024]
    y_rt = y.rearrange(
        "(j t p) (nh n) -> j t nh p n", t=TSUB, p=P, nh=2
    )  # [8,4,2,128,512]
    y_rb = y.rearrange("(j t p) d -> j t p d", t=TSUB, p=P)  # [8,4,128,1024]

    with tile.TileContext(nc) as tc:
        import contextlib

        with contextlib.ExitStack() as ctx:
            wpool = ctx.enter_context(tc.tile_pool(name="wpool", bufs=1))

            # persistent SBUF
            x8_sb = wpool.tile([P, KSUB, S], F8)            # all of x8, 32KB/p
            wqk8_sb = wpool.tile([P, KSUB, 2 * FSH], F8)    # [wq8|wk8]
            wv_sb = wpool.tile([P, KSUB, FSH], BF16)
            wo_sb = wpool.tile([P, FSH // P, D], BF16)
            cinv = wpool.tile([P, SBLK], F32)               # INV const tile
            # per-pair block-diagonal [[KV_h0, 0], [0, KV_h1]] (128x128)
            lhsT2_sb = [
                wpool.tile([P, P], BF16, name=f"l2{p}") for p in range(NPAIR)
            ]
            # per-pair [ksum_h0 x64 | ksum_h1 x64] replicated along free dim
            ksumrep_sb = [
                wpool.tile([P, P], BF16, name=f"kr{p}") for p in range(NPAIR)
            ]

            xbfpool = ctx.enter_context(tc.tile_pool(name="xbf", bufs=4))
            xbfs = []

            # x8 is derived on-chip (DVE cast of the streamed bf16 x, per
            # block) instead of shipped from HBM — saves 4MB of
            # startup-critical DMA. Block 0's x arrives per k-pair
            # interleaved with the wk8 pairs so the K projection starts on
            # the first chunks; everything else queues behind on the
            # in-order sync queue.
            xbf0 = xbfpool.tile([P, KSUB, SBLK], BF16, tag="xbf", name="xbf0")
            xbfs.append(xbf0)
            for i in range(KPAIR):
                nc.sync.dma_start(
                    out=xbf0[:, 2 * i : 2 * i + 2, :],
                    in_=xT_r[:, 2 * i : 2 * i + 2, 0:SBLK],
                )
                nc.sync.dma_start(
                    out=wqk8_sb[:, 2 * i : 2 * i + 2, FSH : 2 * FSH],
                    in_=wqk8_r[:, 2 * i : 2 * i + 2, FSH : 2 * FSH],
                )
            nc.sync.dma_start(out=wv_sb, in_=wv_r)
            nc.sync.dma_start(
                out=wqk8_sb[:, :, 0:FSH], in_=wqk8_r[:, :, 0:FSH]
            )
            nc.sync.dma_start(out=wo_sb, in_=wo_r)
            for j in range(1, NBLK):
                xb = xbfpool.tile([P, KSUB, SBLK], BF16, tag="xbf", name=f"xbf{j}")
                xbfs.append(xb)
                nc.sync.dma_start(
                    out=xb, in_=xT_r[:, :, j * SBLK : (j + 1) * SBLK]
                )

            def cast_x8(j, pair=None):
                # DVE scalar-mult cast bf16 -> fp8e4 with the SX pre-scale
                sl = slice(j * SBLK, (j + 1) * SBLK)
                if pair is None:
                    nc.vector.tensor_scalar_mul(
                        out=x8_sb[:, :, sl], in0=xbfs[j], scalar1=SX
                    )
                else:
                    nc.vector.tensor_scalar_mul(
                        out=x8_sb[:, 2 * pair : 2 * pair + 2, sl],
                        in0=xbfs[j][:, 2 * pair : 2 * pair + 2, :],
                        scalar1=SX,
                    )

            nc.vector.memset(cinv, INV)
            for p_ in range(NPAIR):
                nc.vector.memset(lhsT2_sb[p_], 0.0)
                nc.vector.memset(ksumrep_sb[p_], 0.0)

            # SBUF pools shared across both phases
            etpool = ctx.enter_context(tc.tile_pool(name="et", bufs=3))
            qtpool = ctx.enter_context(tc.tile_pool(name="qt", bufs=2))
            qts = {}

            def qt_elu(ps, j, f):
                # elu(z)+1 = min(exp(z),1) + relu(z), z = ps*INV (fp8
                # descale); Exp/Relu on ACT with the scale pre-op, the
                # combine on DVE
                e = etpool.tile([P, SBLK], F32, tag="e")
                nc.scalar.activation(out=e, in_=ps, func=ACT.Exp, scale=INV)
                r = etpool.tile([P, SBLK], F32, tag="r")
                nc.scalar.activation(out=r, in_=ps, func=ACT.Relu, scale=INV)
                nc.vector.scalar_tensor_tensor(
                    out=qts[j][:, f, :],
                    in0=e,
                    scalar=1.0,
                    in1=r,
                    op0=mybir.AluOpType.min,
                    op1=mybir.AluOpType.add,
                )

            def dr_mm(ps, lhsT, rhs, i):
                nc.tensor.matmul(
                    ps,
                    lhsT,
                    rhs,
                    start=(i == 0),
                    stop=(i == KPAIR - 1),
                    perf_mode=DR,
                )

            # ---------------- phase 1: K,V projection + KV accumulation ----
            with (
                tc.tile_pool(name="kvps", bufs=1, space="PSUM") as kvps_pool,
                tc.tile_pool(name="pa", bufs=4, space="PSUM") as pa_pool,
                tc.tile_pool(name="st", bufs=2) as stpool,
            ):
                kvps = [
                    kvps_pool.tile([P, P + 1], F32, tag=f"kv{p}", name=f"kv{p}")
                    for p in range(NPAIR)
                ]

                bq = []  # lagged [KV | K_sum] accumulation entries

                def emit_b(ent):
                    kst, vst, j, t = ent
                    first = j == 0 and t == 0
                    last = j == NBLK - 1 and t == TSUB - 1
                    for p_ in range(NPAIR):
                        nc.tensor.matmul(
                            kvps[p_],
                            kst[:, t, p_ * P : (p_ + 1) * P],
                            vst[:, t, p_, :],
                            start=first,
                            stop=last,
                        )

                def elu_k(ps, kst, t):
                    # e = exp(ps*INV) on ACT; r = max(ps,0)*INV on DVE via
                    # the cinv const tile; combine min(e,1)+r on DVE
                    e = etpool.tile([P, SBLK], F32, tag="e")
                    nc.scalar.activation(out=e, in_=ps, func=ACT.Exp, scale=INV)
                    r = etpool.tile([P, SBLK], F32, tag="r")
                    nc.vector.scalar_tensor_tensor(
                        out=r,
                        in0=ps,
                        scalar=0.0,
                        in1=cinv,
                        op0=mybir.AluOpType.max,
                        op1=mybir.AluOpType.mult,
                    )
                    nc.vector.scalar_tensor_tensor(
                        out=kst[:, t, :],
                        in0=e,
                        scalar=1.0,
                        in1=r,
                        op0=mybir.AluOpType.min,
                        op1=mybir.AluOpType.add,
                    )

                # block 0: K-sweep k-pair-outer so PE work tracks DMA chunk
                # arrival (4 simultaneous PSUM chains, one per 128-token
                # subtile); V follows t-outer once wv/xbf0 have landed
                kst0 = stpool.tile([P, TSUB, FSH], BF16, tag="kst")
                vst0 = stpool.tile([P, TSUB, NPAIR, P + 1], BF16, tag="vst")
                nc.vector.memset(vst0[:, :, :, P : P + 1], 1.0)
                psks = [
                    pa_pool.tile([P, SBLK], F32, tag="pa", name=f"psk{t}")
                    for t in range(TSUB)
                ]
                for i in range(KPAIR):
                    cast_x8(0, pair=i)
                    for t in range(TSUB):
                        dr_mm(
                            psks[t],
                            x8_sb[:, 2 * i : 2 * i + 2, t * P : (t + 1) * P],
                            wqk8_sb[:, 2 * i : 2 * i + 2, FSH : 2 * FSH],
                            i,
                        )
                for t in range(TSUB):
                    elu_k(psks[t], kst0, t)
                for t in range(TSUB):
                    psv = pa_pool.tile([P, SBLK], F32, tag="pa", name=f"psv{t}")
                    for k in range(KSUB):
                        nc.tensor.matmul(
                            psv,
                            xbf0[:, k, t * P : (t + 1) * P],
                            wv_sb[:, k, :],
                            start=(k == 0),
                            stop=(k == KSUB - 1),
                        )
                    if t >= 1:
                        emit_b(bq.pop(0))
                    nc.scalar.copy(out=vst0[:, t, :, 0:P], in_=psv)
                    bq.append((kst0, vst0, 0, t))

                # blocks 1..7: token-subtile-outer, B lagged one step
                for j in range(1, NBLK):
                    kst = stpool.tile([P, TSUB, FSH], BF16, tag="kst")
                    vst = stpool.tile([P, TSUB, NPAIR, P + 1], BF16, tag="vst")
                    nc.vector.memset(vst[:, :, :, P : P + 1], 1.0)
                    xbf = xbfs[j]
                    cast_x8(j)
                    for t in range(TSUB):
                        tok = j * SBLK + t * P
                        psk = pa_pool.tile([P, SBLK], F32, tag="pa")
                        psv = pa_pool.tile([P, SBLK], F32, tag="pa")
                        for i in range(KPAIR):
                            dr_mm(
                                psk,
                                x8_sb[:, 2 * i : 2 * i + 2, tok : tok + P],
                                wqk8_sb[:, 2 * i : 2 * i + 2, FSH : 2 * FSH],
                                i,
                            )
                        for k in range(KSUB):
                            nc.tensor.matmul(
                                psv,
                                xbf[:, k, t * P : (t + 1) * P],
                                wv_sb[:, k, :],
                                start=(k == 0),
                                stop=(k == KSUB - 1),
                            )
                        emit_b(bq.pop(0))
                        elu_k(psk, kst, t)
                        nc.scalar.copy(out=vst[:, t, :, 0:P], in_=psv)
                        bq.append((kst, vst, j, t))
                # block 0's Q projection runs here, inside the phase-1 PSUM
                # pools: it has no dependency on the KV state, so it keeps
                # the PE busy across the phase boundary
                qts[0] = qtpool.tile([P, NPAIR, SBLK], BF16, tag="qt", name="qt0")
                for f in range(FSH // P):
                    psq = pa_pool.tile([P, SBLK], F32, tag="pa")
                    for i in range(KPAIR):
                        dr_mm(
                            psq,
                            wqk8_sb[:, 2 * i : 2 * i + 2, f * P : (f + 1) * P],
                            x8_sb[:, 2 * i : 2 * i + 2, 0:SBLK],
                            i,
                        )
                    if f == 0:
                        emit_b(bq.pop(0))
                    qt_elu(psq, 0, f)

                # extraction: blockdiag KV + replicated K_sum (zeros preset)
                for p_ in range(NPAIR):
                    nc.vector.tensor_copy(
                        out=lhsT2_sb[p_][0:HD, 0:HD], in_=kvps[p_][0:HD, 0:HD]
                    )
                    nc.vector.tensor_copy(
                        out=lhsT2_sb[p_][HD:P, HD:P], in_=kvps[p_][HD:P, HD:P]
                    )
                    nc.vector.tensor_copy(
                        out=ksumrep_sb[p_][0:HD, 0:HD],
                        in_=kvps[p_][0:HD, P : P + 1].to_broadcast((HD, HD)),
                    )
                    nc.vector.tensor_copy(
                        out=ksumrep_sb[p_][HD:P, HD:P],
                        in_=kvps[p_][HD:P, P : P + 1].to_broadcast((HD, HD)),
                    )

            # ---------------- phase 2: Q projection + attention + Wo -------
            with (
                tc.tile_pool(name="mm512", bufs=3, space="PSUM") as mmps,
                tc.tile_pool(name="pc", bufs=3, space="PSUM") as pcps,
                tc.tile_pool(name="pnb", bufs=2, space="PSUM") as pnps,
                tc.tile_pool(name="ou", bufs=3) as oupool,
                tc.tile_pool(name="rc", bufs=4) as rcpool,
                tc.tile_pool(name="ot", bufs=2) as otpool,
                tc.tile_pool(name="ys", bufs=4) as ypool,
            ):
                outus = {}
                rcbs = {}
                outts = {}

                def qt_half(j, fh):
                    if j not in qts:
                        qts[j] = qtpool.tile(
                            [P, NPAIR, SBLK], BF16, tag="qt", name=f"qt{j}"
                        )
                    for f in (2 * fh, 2 * fh + 1):
                        ps = mmps.tile([P, SBLK], F32, tag="mm")
                        for i in range(KPAIR):
                            dr_mm(
                                ps,
                                wqk8_sb[:, 2 * i : 2 * i + 2, f * P : (f + 1) * P],
                                x8_sb[:, 2 * i : 2 * i + 2, j * SBLK : (j + 1) * SBLK],
                                i,
                            )
                        qt_elu(ps, j, f)

                def attn_pairs(j, pairs):
                    # per pair: attention matmul + broadcast-normalizer
                    # matmul; the DVE apply-multiply reads psc directly from
                    # PSUM (no ACT eviction, one less latency link)
                    qtj = qts[j]
                    if j not in outts:
                        outts[j] = otpool.tile(
                            [P, NPAIR, SBLK], BF16, tag="outt", name="outt"
                        )
                    outt = outts[j]
                    for p_ in pairs:
                        psc = pcps.tile([P, SBLK], F32, tag="pc", name="psc")
                        nc.tensor.matmul(
                            psc,
                            lhsT2_sb[p_],
                            qtj[:, p_, :],
                            start=True,
                            stop=True,
                        )
                        psn = pnps.tile([P, SBLK], F32, tag="pn", name="psn")
                        nc.tensor.matmul(
                            psn,
                            ksumrep_sb[p_],
                            qtj[:, p_, :],
                            start=True,
                            stop=True,
                        )
                        rcb = rcpool.tile([P, SBLK], F32, tag="rcb", name="rcb")
                        nc.vector.reciprocal_approx_fast(out=rcb[:], in_=psn[:])
                        nc.vector.tensor_tensor(
                            out=outt[:, p_, :],
                            in0=psc[:],
                            in1=rcb[:],
                            op=mybir.AluOpType.mult,
                        )

                def psc_section(j):
                    # finale-only variant: ACT-evicts psc to outu so the
                    # apply can be split per token subtile in the drain
                    qtj = qts.pop(j)
                    outu = oupool.tile([P, NPAIR, SBLK], F32, tag="outu")
                    outus[j] = outu
                    rcbs[j] = []
                    for p_ in range(NPAIR):
                        psc = pcps.tile([P, SBLK], F32, tag="pc")
                        nc.tensor.matmul(
                            psc,
                            lhsT2_sb[p_],
                            qtj[:, p_, :],
                            start=True,
                            stop=True,
                        )
                        nc.scalar.copy(out=outu[:, p_, :], in_=psc)
                        psn = pnps.tile([P, SBLK], F32, tag="pn")
                        nc.tensor.matmul(
                            psn,
                            ksumrep_sb[p_],
                            qtj[:, p_, :],
                            start=True,
                            stop=True,
                        )
                        rcb = rcpool.tile([P, SBLK], F32, tag="rcb")
                        nc.vector.reciprocal_approx_fast(out=rcb[:], in_=psn[:])
                        rcbs[j].append(rcb)

                def d_t(j, outt, t, drain=False):
                    ysb = ypool.tile([P, D], F32, tag="ysb", name="ysb")
                    psy0 = mmps.tile([P, 512], F32, tag="mm", name="psy0")
                    psy1 = mmps.tile([P, 512], F32, tag="mm", name="psy1")
                    for fs in range(FSH // P):
                        nc.tensor.matmul(
                            psy0,
                            outt[:, fs, t * P : (t + 1) * P],
                            wo_sb[:, fs, 0:512],
                            start=(fs == 0),
                            stop=(fs == FSH // P - 1),
                        )
                        nc.tensor.matmul(
                            psy1,
                            outt[:, fs, t * P : (t + 1) * P],
                            wo_sb[:, fs, 512:1024],
                            start=(fs == 0),
                            stop=(fs == FSH // P - 1),
                        )
                    if drain:
                        # DVE is otherwise idle in the drain; parallel evict
                        # and split the DMA per half so it starts earlier
                        nc.scalar.copy(out=ysb[:, 0:512], in_=psy0)
                        nc.sync.dma_start(out=y_rt[j, t, 0], in_=ysb[:, 0:512])
                        nc.vector.tensor_copy(out=ysb[:, 512:1024], in_=psy1)
                        nc.sync.dma_start(out=y_rt[j, t, 1], in_=ysb[:, 512:1024])
                    else:
                        nc.scalar.copy(out=ysb[:, 0:512], in_=psy0)
                        nc.scalar.copy(out=ysb[:, 512:1024], in_=psy1)
                        nc.sync.dma_start(out=y_rb[j, t], in_=ysb)

                def d_block(j):
                    outt = outts.pop(j)
                    for t in range(TSUB):
                        d_t(j, outt, t)

                def finale(j):
                    # drain block: apply-multiplies split per token subtile
                    # so each D chain starts as soon as its slice is scaled
                    outt = otpool.tile([P, NPAIR, SBLK], BF16, tag="outt")
                    outu = outus.pop(j)
                    rcs = rcbs.pop(j)
                    for t in range(TSUB):
                        sl = slice(t * P, (t + 1) * P)
                        for p_ in range(NPAIR):
                            nc.vector.tensor_tensor(
                                out=outt[:, p_, sl],
                                in0=outu[:, p_, sl],
                                in1=rcs[p_][:, sl],
                                op=mybir.AluOpType.mult,
                            )
                        d_t(j, outt, t, drain=(t == TSUB - 1))

                # steady-state emission: block j's Q projection brackets
                # block j-1's attention chain so the PE never waits on the
                # ACT/DVE eviction+reciprocal+apply latency.
                for j in range(1, NBLK):
                    attn_pairs(j - 1, [0, 1])
                    attn_pairs(j - 1, [2, 3])
                    qt_half(j, 0)
                    qt_half(j, 1)
                    if j == NBLK - 1:
                        # last block's attention section runs ahead of
                        # d_block(6) so its evictions and reciprocals hide
                        # under D's matmuls and the drain starts immediately
                        psc_section(NBLK - 1)
                    d_block(j - 1)
                finale(NBLK - 1)

    nc.compile()
    return nc


def _prep_inputs(x, Wqkv, Wo):
    import ml_dtypes

    x = np.ascontiguousarray(x, dtype=np.float32)
    Wqkv = np.ascontiguousarray(Wqkv, dtype=np.float32)
    Wo = np.ascontiguousarray(Wo, dtype=np.float32)

    def f8(a):
        return np.clip(a, -240.0, 240.0).astype(ml_dtypes.float8_e4m3fn)

    in_maps = []
    for b in range(B):
        xT = np.ascontiguousarray(x[b].T).astype(ml_dtypes.bfloat16)  # [D, S]
        for hh in range(2):
            cols = slice(hh * FSH, (hh + 1) * FSH)
            wq = Wqkv[:, 0 * D :][:, cols]
            wk = Wqkv[:, 1 * D :][:, cols]
            wv = Wqkv[:, 2 * D :][:, cols]
            wqk8 = f8(
                np.ascontiguousarray(np.concatenate([wq, wk], axis=1)) * SW
            )
            wv_sh = np.ascontiguousarray(wv).astype(ml_dtypes.bfloat16)
            wo_sh = np.ascontiguousarray(Wo[hh * FSH : (hh + 1) * FSH, :]).astype(
                ml_dtypes.bfloat16
            )
            in_maps.append(
                {"xT": xT, "wqk8": wqk8, "wv": wv_sh, "wo": wo_sh}
            )
    return in_maps


def kernel(x, Wqkv, Wo):
    global _NC_CACHE
    if _NC_CACHE is None:
        _NC_CACHE = build()
    nc = _NC_CACHE
    in_maps = _prep_inputs(x, Wqkv, Wo)
    res = run_bass_kernel_spmd(nc, in_maps, list(range(2 * B))).results
    y = np.empty((B, S, D), dtype=np.float32)
    for b in range(B):
        y[b] = res[2 * b]["y"] + res[2 * b + 1]["y"]
    return y


# revision 13
# speedup vs baseline: 1.1808x; 1.0034x over previous
"""Linear attention (B=4, S=4096, D=1024, H=16) on 8 TRN2 NeuronCores.

Sharding: core = (batch, head-half): each core handles one batch's 8 heads.
 - x is host-transposed to xT [D, S] per batch so both operand orientations
   of every matmul come out of the tensor engine with no on-device transpose.
 - Wqkv column-sharded per head-half; Wo row-sharded; host sums the two
   partial y's per batch (row-parallel unshard).

v2: K and Q projections run in fp8e4 DoubleRow (2 contraction rows per PE
cell, ~1.8x per-matmul throughput). Errors in K/Q largely cancel through
the attention normalizer (measured end-to-end ~1.25e-2 vs 2e-2 budget);
V/out-proj paths stay bf16 (their fp8 error flows straight to the output).
Host ships x8 = e4m3(xT*16) and wqk8 = e4m3([Wq|Wk]*512); the 1/8192
descale folds into the ACT activation scale of the elu evaluation, and the
K-path relu becomes a DVE scalar_tensor_tensor multiply with a constant
tile, so ACT/DVE load is unchanged vs the bf16 kernel.

Two-phase dataflow (V/out matmuls bf16, fp32 PSUM accumulate):

phase 1 (per 512-token block): K projection fp8-DR (4 k-pair matmuls per
  128-token subtile) -> elu+1(K); V projection bf16 (8 k matmuls) ->
  [KV | K_sum^T] PSUM accumulation per head-pair (vst carries a ones
  column so one matmul does both). bf16 x is streamed per block (4-deep
  pool) since only the V projection reads it; x8 stays fully resident.
  Block 0 runs the K projection k-pair-outer (4 simultaneous PSUM chains)
  so compute starts as soon as the first (x8, wk8) DMA chunk lands.
  Block 0's Q projection runs at the end of phase 1 to bridge the
  phase transition with PE work that has no KV dependency.

phase 2 (per block, software-pipelined across j):
  QT [512f, 512s] feature-major fp8-DR (lhsT=wq8 pair, rhs=x8 pair) ->
  elu+1 -> bf16
  psc[128,s] = blockdiag(KV_h0, KV_h1)^T @ QT_pair (bf16); norm via the
  replicated-ksum matmul; 1/x on DVE fast reciprocal; outT = outu * rcp
  y[s,:] = outT^T @ Wo per 128-token subtile, fp32, one 512KB DMA per
  subtile.
"""

import numpy as np

import concourse.bacc as bacc
import concourse.mybir as mybir
import concourse.tile as tile
from concourse.bass_utils import run_bass_kernel_spmd

F32 = mybir.dt.float32
BF16 = mybir.dt.bfloat16
F8 = mybir.dt.float8e4
ACT = mybir.ActivationFunctionType
DR = mybir.MatmulPerfMode.DoubleRow

P = 128
B, S, D = 4, 4096, 1024
H = 16
HD = 64

FSH = 512            # features per core for each of Q, K, V (8 heads)
KSUB = D // P        # 8 contraction subtiles
KPAIR = KSUB // 2    # 4 fp8 DoubleRow contraction pairs
SBLK = 512           # tokens per block
NBLK = S // SBLK     # 8 blocks
TSUB = SBLK // P     # 4 token subtiles per block
NPAIR = 4            # head pairs per core

SX = 16.0            # fp8 pre-scale on x
SW = 512.0           # fp8 pre-scale on Wq/Wk
INV = 1.0 / (SX * SW)

_NC_CACHE = None


def build():
    nc = bacc.Bacc(target_bir_lowering=False)
    xT = nc.dram_tensor("xT", [D, S], BF16, kind="ExternalInput")
    wqk8 = nc.dram_tensor("wqk8", [D, 2 * FSH], F8, kind="ExternalInput")
    wv = nc.dram_tensor("wv", [D, FSH], BF16, kind="ExternalInput")
    wo = nc.dram_tensor("wo", [FSH, D], BF16, kind="ExternalInput")
    y = nc.dram_tensor("y", [S, D], F32, kind="ExternalOutput")

    xT_r = xT.rearrange("(ko p) s -> p ko s", p=P)        # [128, 8, 4096]
    wqk8_r = wqk8.rearrange("(ko p) f -> p ko f", p=P)    # [128, 8, 1024]
    wv_r = wv.rearrange("(ko p) f -> p ko f", p=P)        # [128, 8, 512]
    wo_r = wo.rearrange("(fo p) n -> p fo n", p=P)        # [128, 4, 1024]
    y_rt = y.rearrange(
        "(j t p) (nh n) -> j t nh p n", t=TSUB, p=P, nh=2
    )  # [8,4,2,128,512]
    y_rb = y.rearrange("(j t p) d -> j t p d", t=TSUB, p=P)  # [8,4,128,1024]

    with tile.TileContext(nc) as tc:
        import contextlib

        with contextlib.ExitStack() as ctx:
            wpool = ctx.enter_context(tc.tile_pool(name="wpool", bufs=1))

            # persistent SBUF
            x8_sb = wpool.tile([P, KSUB, S], F8)            # all of x8, 32KB/p
            wqk8_sb = wpool.tile([P, KSUB, 2 * FSH], F8)    # [wq8|wk8]
            wv_sb = wpool.tile([P, KSUB, FSH], BF16)
            wo_sb = wpool.tile([P, FSH // P, D], BF16)
            cinv = wpool.tile([P, SBLK], F32)               # INV const tile
            # per-pair block-diagonal [[KV_h0, 0], [0, KV_h1]] (128x128)
            lhsT2_sb = [
                wpool.tile([P, P], BF16, name=f"l2{p}") for p in range(NPAIR)
            ]
            # per-pair [ksum_h0 x64 | ksum_h1 x64] replicated along free dim
            ksumrep_sb = [
                wpool.tile([P, P], BF16, name=f"kr{p}") for p in range(NPAIR)
            ]

            xbfpool = ctx.enter_context(tc.tile_pool(name="xbf", bufs=4))
            xbfs = []

            # x8 is derived on-chip (DVE cast of the streamed bf16 x, per
            # block) instead of shipped from HBM — saves 4MB of
            # startup-critical DMA. Block 0's x arrives per k-pair
            # interleaved with the wk8 pairs so the K projection starts on
            # the first chunks; everything else queues behind on the
            # in-order sync queue.
            xbf0 = xbfpool.tile([P, KSUB, SBLK], BF16, tag="xbf", name="xbf0")
            xbfs.append(xbf0)
            for i in range(KPAIR):
                nc.sync.dma_start(
                    out=xbf0[:, 2 * i : 2 * i + 2, :],
                    in_=xT_r[:, 2 * i : 2 * i + 2, 0:SBLK],
                )
                nc.sync.dma_start(
                    out=wqk8_sb[:, 2 * i : 2 * i + 2, FSH : 2 * FSH],
                    in_=wqk8_r[:, 2 * i : 2 * i + 2, FSH : 2 * FSH],
                )
                nc.sync.dma_start(
                    out=wv_sb[:, 2 * i : 2 * i + 2, :],
                    in_=wv_r[:, 2 * i : 2 * i + 2, :],
                )

            def xbf_dma(j):
                xb = xbfpool.tile([P, KSUB, SBLK], BF16, tag="xbf", name=f"xbf{j}")
                xbfs.append(xb)
                nc.sync.dma_start(
                    out=xb, in_=xT_r[:, :, j * SBLK : (j + 1) * SBLK]
                )

            # wq8/wo are not needed until the very end of phase 1, so they
            # queue behind the first two x prefetches
            xbf_dma(1)
            nc.sync.dma_start(
                out=wqk8_sb[:, :, 0:FSH], in_=wqk8_r[:, :, 0:FSH]
            )
            xbf_dma(2)
            nc.sync.dma_start(out=wo_sb, in_=wo_r)
            for j in range(3, NBLK):
                xbf_dma(j)

            def cast_x8(j, pair=None):
                # DVE scalar-mult cast bf16 -> fp8e4 with the SX pre-scale
                sl = slice(j * SBLK, (j + 1) * SBLK)
                if pair is None:
                    nc.vector.tensor_scalar_mul(
                        out=x8_sb[:, :, sl], in0=xbfs[j], scalar1=SX
                    )
                else:
                    nc.vector.tensor_scalar_mul(
                        out=x8_sb[:, 2 * pair : 2 * pair + 2, sl],
                        in0=xbfs[j][:, 2 * pair : 2 * pair + 2, :],
                        scalar1=SX,
                    )

            nc.vector.memset(cinv, INV)
            for p_ in range(NPAIR):
                nc.vector.memset(lhsT2_sb[p_], 0.0)
                nc.vector.memset(ksumrep_sb[p_], 0.0)

            # SBUF pools shared across both phases
            etpool = ctx.enter_context(tc.tile_pool(name="et", bufs=3))
            qtpool = ctx.enter_context(tc.tile_pool(name="qt", bufs=2))
            qts = {}

            def qt_elu(ps, j, f):
                # elu(z)+1 = min(exp(z),1) + relu(z), z = ps*INV (fp8
                # descale); Exp/Relu on ACT with the scale pre-op, the
                # combine on DVE
                e = etpool.tile([P, SBLK], F32, tag="e")
                nc.scalar.activation(out=e, in_=ps, func=ACT.Exp, scale=INV)
                r = etpool.tile([P, SBLK], F32, tag="r")
                nc.scalar.activation(out=r, in_=ps, func=ACT.Relu, scale=INV)
                nc.vector.scalar_tensor_tensor(
                    out=qts[j][:, f, :],
                    in0=e,
                    scalar=1.0,
                    in1=r,
                    op0=mybir.AluOpType.min,
                    op1=mybir.AluOpType.add,
                )

            def dr_mm(ps, lhsT, rhs, i):
                nc.tensor.matmul(
                    ps,
                    lhsT,
                    rhs,
                    start=(i == 0),
                    stop=(i == KPAIR - 1),
                    perf_mode=DR,
                )

            # ---------------- phase 1: K,V projection + KV accumulation ----
            with (
                tc.tile_pool(name="kvps", bufs=1, space="PSUM") as kvps_pool,
                tc.tile_pool(name="pa", bufs=4, space="PSUM") as pa_pool,
                tc.tile_pool(name="st", bufs=2) as stpool,
            ):
                kvps = [
                    kvps_pool.tile([P, P + 1], F32, tag=f"kv{p}", name=f"kv{p}")
                    for p in range(NPAIR)
                ]

                bq = []  # lagged [KV | K_sum] accumulation entries

                def emit_b(ent):
                    kst, vst, j, t = ent
                    first = j == 0 and t == 0
                    last = j == NBLK - 1 and t == TSUB - 1
                    for p_ in range(NPAIR):
                        nc.tensor.matmul(
                            kvps[p_],
                            kst[:, t, p_ * P : (p_ + 1) * P],
                            vst[:, t, p_, :],
                            start=first,
                            stop=last,
                        )

                def elu_k(ps, kst, t):
                    # e = exp(ps*INV) on ACT; r = max(ps,0)*INV on DVE via
                    # the cinv const tile; combine min(e,1)+r on DVE
                    e = etpool.tile([P, SBLK], F32, tag="e")
                    nc.scalar.activation(out=e, in_=ps, func=ACT.Exp, scale=INV)
                    r = etpool.tile([P, SBLK], F32, tag="r")
                    nc.vector.scalar_tensor_tensor(
                        out=r,
                        in0=ps,
                        scalar=0.0,
                        in1=cinv,
                        op0=mybir.AluOpType.max,
                        op1=mybir.AluOpType.mult,
                    )
                    nc.vector.scalar_tensor_tensor(
                        out=kst[:, t, :],
                        in0=e,
                        scalar=1.0,
                        in1=r,
                        op0=mybir.AluOpType.min,
                        op1=mybir.AluOpType.add,
                    )

                # block 0: K-sweep k-pair-outer so PE work tracks DMA chunk
                # arrival (4 simultaneous PSUM chains, one per 128-token
                # subtile); V follows t-outer once wv/xbf0 have landed
                kst0 = stpool.tile([P, TSUB, FSH], BF16, tag="kst")
                vst0 = stpool.tile([P, TSUB, NPAIR, P + 1], BF16, tag="vst")
                nc.vector.memset(vst0[:, :, :, P : P + 1], 1.0)
                psks = [
                    pa_pool.tile([P, SBLK], F32, tag="pa", name=f"psk{t}")
                    for t in range(TSUB)
                ]
                for i in range(KPAIR):
                    cast_x8(0, pair=i)
                    for t in range(TSUB):
                        dr_mm(
                            psks[t],
                            x8_sb[:, 2 * i : 2 * i + 2, t * P : (t + 1) * P],
                            wqk8_sb[:, 2 * i : 2 * i + 2, FSH : 2 * FSH],
                            i,
                        )
                for t in range(TSUB):
                    elu_k(psks[t], kst0, t)
                for t in range(TSUB):
                    psv = pa_pool.tile([P, SBLK], F32, tag="pa", name=f"psv{t}")
                    for k in range(KSUB):
                        nc.tensor.matmul(
                            psv,
                            xbf0[:, k, t * P : (t + 1) * P],
                            wv_sb[:, k, :],
                            start=(k == 0),
                            stop=(k == KSUB - 1),
                        )
                    if t >= 1:
                        emit_b(bq.pop(0))
                    nc.scalar.copy(out=vst0[:, t, :, 0:P], in_=psv)
                    bq.append((kst0, vst0, 0, t))

                # blocks 1..7: token-subtile-outer, B lagged one step
                for j in range(1, NBLK):
                    kst = stpool.tile([P, TSUB, FSH], BF16, tag="kst")
                    vst = stpool.tile([P, TSUB, NPAIR, P + 1], BF16, tag="vst")
                    nc.vector.memset(vst[:, :, :, P : P + 1], 1.0)
                    xbf = xbfs[j]
                    cast_x8(j)
                    for t in range(TSUB):
                        tok = j * SBLK + t * P
                        psk = pa_pool.tile([P, SBLK], F32, tag="pa")
                        psv = pa_pool.tile([P, SBLK], F32, tag="pa")
                        for i in range(KPAIR):
                            dr_mm(
                                psk,
                                x8_sb[:, 2 * i : 2 * i + 2, tok : tok + P],
                                wqk8_sb[:, 2 * i : 2 * i + 2, FSH : 2 * FSH],
                                i,
                            )
                        for k in range(KSUB):
                            nc.tensor.matmul(
                                psv,
                                xbf[:, k, t * P : (t + 1) * P],
                                wv_sb[:, k, :],
                                start=(k == 0),
                                stop=(k == KSUB - 1),
                            )
                        emit_b(bq.pop(0))
                        elu_k(psk, kst, t)
                        nc.scalar.copy(out=vst[:, t, :, 0:P], in_=psv)
                        bq.append((kst, vst, j, t))
                # block 0's Q projection runs here, inside the phase-1 PSUM
                # pools: it has no dependency on the KV state, so it keeps
                # the PE busy across the phase boundary
                qts[0] = qtpool.tile([P, NPAIR, SBLK], BF16, tag="qt", name="qt0")
                for f in range(FSH // P):
                    psq = pa_pool.tile([P, SBLK], F32, tag="pa")
                    for i in range(KPAIR):
                        dr_mm(
                            psq,
                            wqk8_sb[:, 2 * i : 2 * i + 2, f * P : (f + 1) * P],
                            x8_sb[:, 2 * i : 2 * i + 2, 0:SBLK],
                            i,
                        )
                    if f == 0:
                        emit_b(bq.pop(0))
                        # extraction right after the final KV accumulation:
                        # the DVE copies hide under the remaining Q-chain
                        # matmuls instead of delaying phase 2's first psc
                        for p_ in range(NPAIR):
                            nc.vector.tensor_copy(
                                out=lhsT2_sb[p_][0:HD, 0:HD],
                                in_=kvps[p_][0:HD, 0:HD],
                            )
                            nc.vector.tensor_copy(
                                out=lhsT2_sb[p_][HD:P, HD:P],
                                in_=kvps[p_][HD:P, HD:P],
                            )
                            nc.vector.tensor_copy(
                                out=ksumrep_sb[p_][0:HD, 0:HD],
                                in_=kvps[p_][0:HD, P : P + 1].to_broadcast(
                                    (HD, HD)
                                ),
                            )
                            nc.vector.tensor_copy(
                                out=ksumrep_sb[p_][HD:P, HD:P],
                                in_=kvps[p_][HD:P, P : P + 1].to_broadcast(
                                    (HD, HD)
                                ),
                            )
                    qt_elu(psq, 0, f)

            # ---------------- phase 2: Q projection + attention + Wo -------
            with (
                tc.tile_pool(name="mm512", bufs=3, space="PSUM") as mmps,
                tc.tile_pool(name="pc", bufs=3, space="PSUM") as pcps,
                tc.tile_pool(name="pnb", bufs=2, space="PSUM") as pnps,
                tc.tile_pool(name="ou", bufs=3) as oupool,
                tc.tile_pool(name="rc", bufs=4) as rcpool,
                tc.tile_pool(name="ot", bufs=2) as otpool,
                tc.tile_pool(name="ys", bufs=4) as ypool,
            ):
                outus = {}
                rcbs = {}
                outts = {}

                def qt_half(j, fh):
                    if j not in qts:
                        qts[j] = qtpool.tile(
                            [P, NPAIR, SBLK], BF16, tag="qt", name=f"qt{j}"
                        )
                    for f in (2 * fh, 2 * fh + 1):
                        ps = mmps.tile([P, SBLK], F32, tag="mm")
                        for i in range(KPAIR):
                            dr_mm(
                                ps,
                                wqk8_sb[:, 2 * i : 2 * i + 2, f * P : (f + 1) * P],
                                x8_sb[:, 2 * i : 2 * i + 2, j * SBLK : (j + 1) * SBLK],
                                i,
                            )
                        qt_elu(ps, j, f)

                def attn_pairs(j, pairs):
                    # per pair: attention matmul + broadcast-normalizer
                    # matmul; the DVE apply-multiply reads psc directly from
                    # PSUM (no ACT eviction, one less latency link)
                    qtj = qts[j]
                    if j not in outts:
                        outts[j] = otpool.tile(
                            [P, NPAIR, SBLK], BF16, tag="outt", name="outt"
                        )
                    outt = outts[j]
                    for p_ in pairs:
                        psc = pcps.tile([P, SBLK], F32, tag="pc", name="psc")
                        nc.tensor.matmul(
                            psc,
                            lhsT2_sb[p_],
                            qtj[:, p_, :],
                            start=True,
                            stop=True,
                        )
                        psn = pnps.tile([P, SBLK], F32, tag="pn", name="psn")
                        nc.tensor.matmul(
                            psn,
                            ksumrep_sb[p_],
                            qtj[:, p_, :],
                            start=True,
                            stop=True,
                        )
                        rcb = rcpool.tile([P, SBLK], F32, tag="rcb", name="rcb")
                        nc.vector.reciprocal_approx_fast(out=rcb[:], in_=psn[:])
                        nc.vector.tensor_tensor(
                            out=outt[:, p_, :],
                            in0=psc[:],
                            in1=rcb[:],
                            op=mybir.AluOpType.mult,
                        )

                def psc_section(j):
                    # finale-only variant: ACT-evicts psc to outu so the
                    # apply can be split per token subtile in the drain
                    qtj = qts.pop(j)
                    outu = oupool.tile([P, NPAIR, SBLK], F32, tag="outu")
                    outus[j] = outu
                    rcbs[j] = []
                    for p_ in range(NPAIR):
                        psc = pcps.tile([P, SBLK], F32, tag="pc")
                        nc.tensor.matmul(
                            psc,
                            lhsT2_sb[p_],
                            qtj[:, p_, :],
                            start=True,
                            stop=True,
                        )
                        nc.scalar.copy(out=outu[:, p_, :], in_=psc)
                        psn = pnps.tile([P, SBLK], F32, tag="pn")
                        nc.tensor.matmul(
                            psn,
                            ksumrep_sb[p_],
                            qtj[:, p_, :],
                            start=True,
                            stop=True,
                        )
                        rcb = rcpool.tile([P, SBLK], F32, tag="rcb")
                        nc.vector.reciprocal_approx_fast(out=rcb[:], in_=psn[:])
                        rcbs[j].append(rcb)

                def d_t(j, outt, t, drain=False):
                    ysb = ypool.tile([P, D], F32, tag="ysb", name="ysb")
                    psy0 = mmps.tile([P, 512], F32, tag="mm", name="psy0")
                    psy1 = mmps.tile([P, 512], F32, tag="mm", name="psy1")
                    for fs in range(FSH // P):
                        nc.tensor.matmul(
                            psy0,
                            outt[:, fs, t * P : (t + 1) * P],
                            wo_sb[:, fs, 0:512],
                            start=(fs == 0),
                            stop=(fs == FSH // P - 1),
                        )
                        nc.tensor.matmul(
                            psy1,
                            outt[:, fs, t * P : (t + 1) * P],
                            wo_sb[:, fs, 512:1024],
                            start=(fs == 0),
                            stop=(fs == FSH // P - 1),
                        )
                    if drain:
                        # split the DMA per half so it starts earlier
                        nc.scalar.copy(out=ysb[:, 0:512], in_=psy0)
                        nc.sync.dma_start(out=y_rt[j, t, 0], in_=ysb[:, 0:512])
                        nc.vector.tensor_copy(out=ysb[:, 512:1024], in_=psy1)
                        nc.sync.dma_start(out=y_rt[j, t, 1], in_=ysb[:, 512:1024])
                    else:
                        # one eviction on ACT, one on DVE: phase-2 ACT is
                        # otherwise the second-busiest engine behind the PE
                        nc.scalar.copy(out=ysb[:, 0:512], in_=psy0)
                        nc.vector.tensor_copy(out=ysb[:, 512:1024], in_=psy1)
                        nc.sync.dma_start(out=y_rb[j, t], in_=ysb)

                def d_half(j, ts):
                    for t in ts:
                        d_t(j, outts[j], t)

                def finale(j):
                    # drain block: apply-multiplies split per token subtile
                    # so each D chain starts as soon as its slice is scaled
                    outt = otpool.tile([P, NPAIR, SBLK], BF16, tag="outt")
                    outu = outus.pop(j)
                    rcs = rcbs.pop(j)
                    for t in range(TSUB):
                        sl = slice(t * P, (t + 1) * P)
                        for p_ in range(NPAIR):
                            nc.vector.tensor_tensor(
                                out=outt[:, p_, sl],
                                in0=outu[:, p_, sl],
                                in1=rcs[p_][:, sl],
                                op=mybir.AluOpType.mult,
                            )
                        d_t(j, outt, t, drain=(t == TSUB - 1))

                # steady-state emission: block j's Q projection brackets
                # block j-1's attention chain so the PE never waits on the
                # ACT/DVE eviction+reciprocal+apply latency; the out-proj
                # subtiles interleave between the Q halves so block
                # boundaries never wait on the last qt elu.
                for j in range(1, NBLK):
                    attn_pairs(j - 1, [0, 1])
                    attn_pairs(j - 1, [2, 3])
                    qt_half(j, 0)
                    d_half(j - 1, [0, 1])
                    qt_half(j, 1)
                    if j == NBLK - 1:
                        # last block's attention section runs ahead of the
                        # final d_t's so its evictions and reciprocals hide
                        # under D's matmuls and the drain starts immediately
                        psc_section(NBLK - 1)
                    d_half(j - 1, [2, 3])
                    outts.pop(j - 1)
                finale(NBLK - 1)

    nc.compile()
    return nc


def _prep_inputs(x, Wqkv, Wo):
    import ml_dtypes

    x = np.ascontiguousarray(x, dtype=np.float32)
    Wqkv = np.ascontiguousarray(Wqkv, dtype=np.float32)
    Wo = np.ascontiguousarray(Wo, dtype=np.float32)

    def f8(a):
        return np.clip(a, -240.0, 240.0).astype(ml_dtypes.float8_e4m3fn)

    in_maps = []
    for b in range(B):
        xT = np.ascontiguousarray(x[b].T).astype(ml_dtypes.bfloat16)  # [D, S]
        for hh in range(2):
            cols = slice(hh * FSH, (hh + 1) * FSH)
            wq = Wqkv[:, 0 * D :][:, cols]
            wk = Wqkv[:, 1 * D :][:, cols]
            wv = Wqkv[:, 2 * D :][:, cols]
            wqk8 = f8(
                np.ascontiguousarray(np.concatenate([wq, wk], axis=1)) * SW
            )
            wv_sh = np.ascontiguousarray(wv).astype(ml_dtypes.bfloat16)
            wo_sh = np.ascontiguousarray(Wo[hh * FSH : (hh + 1) * FSH, :]).astype(
                ml_dtypes.bfloat16
            )
            in_maps.append(
                {"xT": xT, "wqk8": wqk8, "wv": wv_sh, "wo": wo_sh}
            )
    return in_maps


def kernel(x, Wqkv, Wo):
    global _NC_CACHE
    if _NC_CACHE is None:
        _NC_CACHE = build()
    nc = _NC_CACHE
    in_maps = _prep_inputs(x, Wqkv, Wo)
    res = run_bass_kernel_spmd(nc, in_maps, list(range(2 * B))).results
    y = np.empty((B, S, D), dtype=np.float32)
    for b in range(B):
        y[b] = res[2 * b]["y"] + res[2 * b + 1]["y"]
    return y


# revision 18
# speedup vs baseline: 1.1868x; 1.0051x over previous
"""Linear attention (B=4, S=4096, D=1024, H=16) on 8 TRN2 NeuronCores.

Sharding: core = (batch, head-half): each core handles one batch's 8 heads.
 - x is host-transposed to xT [D, S] per batch so both operand orientations
   of every matmul come out of the tensor engine with no on-device transpose.
 - Wqkv column-sharded per head-half; Wo row-sharded; host sums the two
   partial y's per batch (row-parallel unshard).

v2: K and Q projections run in fp8e4 DoubleRow (2 contraction rows per PE
cell, ~1.8x per-matmul throughput). Errors in K/Q largely cancel through
the attention normalizer (measured end-to-end ~1.25e-2 vs 2e-2 budget);
V/out-proj paths stay bf16 (their fp8 error flows straight to the output).
Host ships x8 = e4m3(xT*16) and wqk8 = e4m3([Wq|Wk]*512); the 1/8192
descale folds into the ACT activation scale of the elu evaluation, and the
K-path relu becomes a DVE scalar_tensor_tensor multiply with a constant
tile, so ACT/DVE load is unchanged vs the bf16 kernel.

Two-phase dataflow (V/out matmuls bf16, fp32 PSUM accumulate):

phase 1 (per 512-token block): K projection fp8-DR (4 k-pair matmuls per
  128-token subtile) -> elu+1(K); V projection bf16 (8 k matmuls) ->
  [KV | K_sum^T] PSUM accumulation per head-pair (vst carries a ones
  column so one matmul does both). bf16 x is streamed per block (4-deep
  pool) since only the V projection reads it; x8 stays fully resident.
  Block 0 runs the K projection k-pair-outer (4 simultaneous PSUM chains)
  so compute starts as soon as the first (x8, wk8) DMA chunk lands.
  Block 0's Q projection runs at the end of phase 1 to bridge the
  phase transition with PE work that has no KV dependency.

phase 2 (per block, software-pipelined across j):
  QT [512f, 512s] feature-major fp8-DR (lhsT=wq8 pair, rhs=x8 pair) ->
  elu+1 -> bf16
  psc[128,s] = blockdiag(KV_h0, KV_h1)^T @ QT_pair (bf16); norm via the
  replicated-ksum matmul; 1/x on DVE fast reciprocal; outT = outu * rcp
  y[s,:] = outT^T @ Wo per 128-token subtile, fp32, one 512KB DMA per
  subtile.
"""

import numpy as np

import concourse.bacc as bacc
import concourse.mybir as mybir
import concourse.tile as tile
from concourse.bass_utils import run_bass_kernel_spmd

F32 = mybir.dt.float32
BF16 = mybir.dt.bfloat16
F8 = mybir.dt.float8e4
ACT = mybir.ActivationFunctionType
DR = mybir.MatmulPerfMode.DoubleRow

P = 128
B, S, D = 4, 4096, 1024
H = 16
HD = 64

FSH = 512            # features per core for each of Q, K, V (8 heads)
KSUB = D // P        # 8 contraction subtiles
KPAIR = KSUB // 2    # 4 fp8 DoubleRow contraction pairs
SBLK = 512           # tokens per block
NBLK = S // SBLK     # 8 blocks
TSUB = SBLK // P     # 4 token subtiles per block
NPAIR = 4            # head pairs per core

SX = 16.0            # fp8 pre-scale on x
SW = 512.0           # fp8 pre-scale on Wq/Wk
INV = 1.0 / (SX * SW)

_NC_CACHE = None


def build():
    nc = bacc.Bacc(target_bir_lowering=False)
    xT = nc.dram_tensor("xT", [D, S], BF16, kind="ExternalInput")
    wqk8 = nc.dram_tensor("wqk8", [D, 2 * FSH], F8, kind="ExternalInput")
    wv = nc.dram_tensor("wv", [D, FSH], BF16, kind="ExternalInput")
    wo = nc.dram_tensor("wo", [FSH, D], BF16, kind="ExternalInput")
    y = nc.dram_tensor("y", [S, D], F32, kind="ExternalOutput")

    xT_r = xT.rearrange("(ko p) s -> p ko s", p=P)        # [128, 8, 4096]
    wqk8_r = wqk8.rearrange("(ko p) f -> p ko f", p=P)    # [128, 8, 1024]
    wv_r = wv.rearrange("(ko p) f -> p ko f", p=P)        # [128, 8, 512]
    wo_r = wo.rearrange("(fo p) n -> p fo n", p=P)        # [128, 4, 1024]
    y_rt = y.rearrange(
        "(j t p) (nh n) -> j t nh p n", t=TSUB, p=P, nh=2
    )  # [8,4,2,128,512]
    y_rb = y.rearrange("(j t p) d -> j t p d", t=TSUB, p=P)  # [8,4,128,1024]

    with tile.TileContext(nc) as tc:
        import contextlib

        with contextlib.ExitStack() as ctx:
            wpool = ctx.enter_context(tc.tile_pool(name="wpool", bufs=1))

            # persistent SBUF
            x8_sb = wpool.tile([P, KSUB, S], F8)            # all of x8, 32KB/p
            wqk8_sb = wpool.tile([P, KSUB, 2 * FSH], F8)    # [wq8|wk8]
            wv_sb = wpool.tile([P, KSUB, FSH], BF16)
            wo_sb = wpool.tile([P, FSH // P, D], BF16)
            cinv = wpool.tile([P, SBLK], F32)               # INV const tile
            # per-pair block-diagonal [[KV_h0, 0], [0, KV_h1]] (128x128)
            lhsT2_sb = [
                wpool.tile([P, P], BF16, name=f"l2{p}") for p in range(NPAIR)
            ]
            # per-pair [ksum_h0 x64 | ksum_h1 x64] replicated along free dim
            ksumrep_sb = [
                wpool.tile([P, P], BF16, name=f"kr{p}") for p in range(NPAIR)
            ]

            xbfpool = ctx.enter_context(tc.tile_pool(name="xbf", bufs=4))
            xbfs = []

            # x8 is derived on-chip (DVE cast of the streamed bf16 x, per
            # block) instead of shipped from HBM — saves 4MB of
            # startup-critical DMA. Block 0's x arrives per k-pair
            # interleaved with the wk8 pairs so the K projection starts on
            # the first chunks; everything else queues behind on the
            # in-order sync queue.
            xbf0 = xbfpool.tile([P, KSUB, SBLK], BF16, tag="xbf", name="xbf0")
            xbfs.append(xbf0)
            for i in range(KPAIR):
                nc.sync.dma_start(
                    out=xbf0[:, 2 * i : 2 * i + 2, :],
                    in_=xT_r[:, 2 * i : 2 * i + 2, 0:SBLK],
                )
                nc.sync.dma_start(
                    out=wqk8_sb[:, 2 * i : 2 * i + 2, FSH : 2 * FSH],
                    in_=wqk8_r[:, 2 * i : 2 * i + 2, FSH : 2 * FSH],
                )
                nc.sync.dma_start(
                    out=wv_sb[:, 2 * i : 2 * i + 2, :],
                    in_=wv_r[:, 2 * i : 2 * i + 2, :],
                )

            def xbf_dma(j):
                xb = xbfpool.tile([P, KSUB, SBLK], BF16, tag="xbf", name=f"xbf{j}")
                xbfs.append(xb)
                nc.sync.dma_start(
                    out=xb, in_=xT_r[:, :, j * SBLK : (j + 1) * SBLK]
                )

            # wq8/wo are not needed until the very end of phase 1, so they
            # queue behind the first two x prefetches
            xbf_dma(1)
            nc.sync.dma_start(
                out=wqk8_sb[:, :, 0:FSH], in_=wqk8_r[:, :, 0:FSH]
            )
            xbf_dma(2)
            nc.sync.dma_start(out=wo_sb, in_=wo_r)
            for j in range(3, NBLK):
                xbf_dma(j)

            def cast_x8(j, pair=None):
                # DVE scalar-mult cast bf16 -> fp8e4 with the SX pre-scale
                sl = slice(j * SBLK, (j + 1) * SBLK)
                if pair is None:
                    nc.vector.tensor_scalar_mul(
                        out=x8_sb[:, :, sl], in0=xbfs[j], scalar1=SX
                    )
                else:
                    nc.vector.tensor_scalar_mul(
                        out=x8_sb[:, 2 * pair : 2 * pair + 2, sl],
                        in0=xbfs[j][:, 2 * pair : 2 * pair + 2, :],
                        scalar1=SX,
                    )

            nc.vector.memset(cinv, INV)
            for p_ in range(NPAIR):
                nc.vector.memset(lhsT2_sb[p_], 0.0)
                nc.vector.memset(ksumrep_sb[p_], 0.0)

            # SBUF pools shared across both phases
            etpool = ctx.enter_context(tc.tile_pool(name="et", bufs=3))
            qtpool = ctx.enter_context(tc.tile_pool(name="qt", bufs=2))
            qts = {}

            def qt_elu(ps, j, f):
                # elu(z)+1 = min(exp(z),1) + relu(z), z = ps*INV (fp8
                # descale); Exp/Relu on ACT with the scale pre-op, the
                # combine on DVE
                e = etpool.tile([P, SBLK], F32, tag="e")
                nc.scalar.activation(out=e, in_=ps, func=ACT.Exp, scale=INV)
                r = etpool.tile([P, SBLK], F32, tag="r")
                nc.scalar.activation(out=r, in_=ps, func=ACT.Relu, scale=INV)
                nc.vector.scalar_tensor_tensor(
                    out=qts[j][:, f, :],
                    in0=e,
                    scalar=1.0,
                    in1=r,
                    op0=mybir.AluOpType.min,
                    op1=mybir.AluOpType.add,
                )

            def dr_mm(ps, lhsT, rhs, i):
                nc.tensor.matmul(
                    ps,
                    lhsT,
                    rhs,
                    start=(i == 0),
                    stop=(i == KPAIR - 1),
                    perf_mode=DR,
                )

            # ---------------- phase 1: K,V projection + KV accumulation ----
            with (
                tc.tile_pool(name="kvps", bufs=1, space="PSUM") as kvps_pool,
                tc.tile_pool(name="pa", bufs=4, space="PSUM") as pa_pool,
                tc.tile_pool(name="st", bufs=2) as stpool,
            ):
                kvps = [
                    kvps_pool.tile([P, P + 1], F32, tag=f"kv{p}", name=f"kv{p}")
                    for p in range(NPAIR)
                ]

                bq = []  # lagged [KV | K_sum] accumulation entries

                def emit_b(ent):
                    kst, vst, j, t = ent
                    first = j == 0 and t == 0
                    last = j == NBLK - 1 and t == TSUB - 1
                    for p_ in range(NPAIR):
                        nc.tensor.matmul(
                            kvps[p_],
                            kst[:, t, p_ * P : (p_ + 1) * P],
                            vst[:, t, p_, :],
                            start=first,
                            stop=last,
                        )

                def elu_k(ps, kst, t):
                    # e = exp(ps*INV) on ACT; r = max(ps,0)*INV on DVE via
                    # the cinv const tile; combine min(e,1)+r on DVE
                    e = etpool.tile([P, SBLK], F32, tag="e")
                    nc.scalar.activation(out=e, in_=ps, func=ACT.Exp, scale=INV)
                    r = etpool.tile([P, SBLK], F32, tag="r")
                    nc.vector.scalar_tensor_tensor(
                        out=r,
                        in0=ps,
                        scalar=0.0,
                        in1=cinv,
                        op0=mybir.AluOpType.max,
                        op1=mybir.AluOpType.mult,
                    )
                    nc.vector.scalar_tensor_tensor(
                        out=kst[:, t, :],
                        in0=e,
                        scalar=1.0,
                        in1=r,
                        op0=mybir.AluOpType.min,
                        op1=mybir.AluOpType.add,
                    )

                # block 0: K-sweep k-pair-outer so PE work tracks DMA chunk
                # arrival (4 simultaneous PSUM chains, one per 128-token
                # subtile); V follows t-outer once wv/xbf0 have landed
                kst0 = stpool.tile([P, TSUB, FSH], BF16, tag="kst")
                vst0 = stpool.tile([P, TSUB, NPAIR, P + 1], BF16, tag="vst")
                nc.vector.memset(vst0[:, :, :, P : P + 1], 1.0)
                psks = [
                    pa_pool.tile([P, SBLK], F32, tag="pa", name=f"psk{t}")
                    for t in range(TSUB)
                ]
                for i in range(KPAIR):
                    cast_x8(0, pair=i)
                    for t in range(TSUB):
                        dr_mm(
                            psks[t],
                            x8_sb[:, 2 * i : 2 * i + 2, t * P : (t + 1) * P],
                            wqk8_sb[:, 2 * i : 2 * i + 2, FSH : 2 * FSH],
                            i,
                        )
                for t in range(TSUB):
                    elu_k(psks[t], kst0, t)
                for t in range(TSUB):
                    psv = pa_pool.tile([P, SBLK], F32, tag="pa", name=f"psv{t}")
                    for k in range(KSUB):
                        nc.tensor.matmul(
                            psv,
                            xbf0[:, k, t * P : (t + 1) * P],
                            wv_sb[:, k, :],
                            start=(k == 0),
                            stop=(k == KSUB - 1),
                        )
                    if t == 0:
                        cast_x8(1)
                    if t >= 1:
                        emit_b(bq.pop(0))
                    nc.scalar.copy(out=vst0[:, t, :, 0:P], in_=psv)
                    bq.append((kst0, vst0, 0, t))

                # blocks 1..7: token-subtile-outer, B lagged one step
                for j in range(1, NBLK):
                    kst = stpool.tile([P, TSUB, FSH], BF16, tag="kst")
                    vst = stpool.tile([P, TSUB, NPAIR, P + 1], BF16, tag="vst")
                    nc.vector.memset(vst[:, :, :, P : P + 1], 1.0)
                    xbf = xbfs[j]
                    if j + 1 < NBLK:
                        # prefetch next block's fp8 cast so it is never on
                        # the K-projection critical path
                        cast_x8(j + 1)
                    for t in range(TSUB):
                        tok = j * SBLK + t * P
                        psk = pa_pool.tile([P, SBLK], F32, tag="pa")
                        psv = pa_pool.tile([P, SBLK], F32, tag="pa")
                        for i in range(KPAIR):
                            dr_mm(
                                psk,
                                x8_sb[:, 2 * i : 2 * i + 2, tok : tok + P],
                                wqk8_sb[:, 2 * i : 2 * i + 2, FSH : 2 * FSH],
                                i,
                            )
                        for k in range(KSUB):
                            nc.tensor.matmul(
                                psv,
                                xbf[:, k, t * P : (t + 1) * P],
                                wv_sb[:, k, :],
                                start=(k == 0),
                                stop=(k == KSUB - 1),
                            )
                        emit_b(bq.pop(0))
                        elu_k(psk, kst, t)
                        nc.scalar.copy(out=vst[:, t, :, 0:P], in_=psv)
                        bq.append((kst, vst, j, t))
                # block 0's Q projection runs here, inside the phase-1 PSUM
                # pools: it has no dependency on the KV state, so it keeps
                # the PE busy across the phase boundary
                qts[0] = qtpool.tile([P, NPAIR, SBLK], BF16, tag="qt", name="qt0")
                for f in range(FSH // P):
                    psq = pa_pool.tile([P, SBLK], F32, tag="pa")
                    for i in range(KPAIR):
                        dr_mm(
                            psq,
                            wqk8_sb[:, 2 * i : 2 * i + 2, f * P : (f + 1) * P],
                            x8_sb[:, 2 * i : 2 * i + 2, 0:SBLK],
                            i,
                        )
                    if f == 0:
                        emit_b(bq.pop(0))
                        # extraction right after the final KV accumulation:
                        # the DVE copies hide under the remaining Q-chain
                        # matmuls instead of delaying phase 2's first psc
                        for p_ in range(NPAIR):
                            nc.vector.tensor_copy(
                                out=lhsT2_sb[p_][0:HD, 0:HD],
                                in_=kvps[p_][0:HD, 0:HD],
                            )
                            nc.vector.tensor_copy(
                                out=lhsT2_sb[p_][HD:P, HD:P],
                                in_=kvps[p_][HD:P, HD:P],
                            )
                            nc.vector.tensor_copy(
                                out=ksumrep_sb[p_][0:HD, 0:HD],
                                in_=kvps[p_][0:HD, P : P + 1].to_broadcast(
                                    (HD, HD)
                                ),
                            )
                            nc.vector.tensor_copy(
                                out=ksumrep_sb[p_][HD:P, HD:P],
                                in_=kvps[p_][HD:P, P : P + 1].to_broadcast(
                                    (HD, HD)
                                ),
                            )
                    qt_elu(psq, 0, f)

            # ---------------- phase 2: Q projection + attention + Wo -------
            with (
                tc.tile_pool(name="mm512", bufs=3, space="PSUM") as mmps,
                tc.tile_pool(name="pc", bufs=3, space="PSUM") as pcps,
                tc.tile_pool(name="pnb", bufs=2, space="PSUM") as pnps,
                tc.tile_pool(name="ou", bufs=3) as oupool,
                tc.tile_pool(name="rc", bufs=4) as rcpool,
                tc.tile_pool(name="ot", bufs=2) as otpool,
                tc.tile_pool(name="ys", bufs=4) as ypool,
            ):
                outus = {}
                rcbs = {}
                outts = {}

                def qt_half(j, fh):
                    if j not in qts:
                        qts[j] = qtpool.tile(
                            [P, NPAIR, SBLK], BF16, tag="qt", name=f"qt{j}"
                        )
                    for f in (2 * fh, 2 * fh + 1):
                        ps = mmps.tile([P, SBLK], F32, tag="mm")
                        for i in range(KPAIR):
                            dr_mm(
                                ps,
                                wqk8_sb[:, 2 * i : 2 * i + 2, f * P : (f + 1) * P],
                                x8_sb[:, 2 * i : 2 * i + 2, j * SBLK : (j + 1) * SBLK],
                                i,
                            )
                        qt_elu(ps, j, f)

                def attn_pairs(j, pairs):
                    # per pair: attention matmul + broadcast-normalizer
                    # matmul; the DVE apply-multiply reads psc directly from
                    # PSUM (no ACT eviction, one less latency link)
                    qtj = qts[j]
                    if j not in outts:
                        outts[j] = otpool.tile(
                            [P, NPAIR, SBLK], BF16, tag="outt", name="outt"
                        )
                    outt = outts[j]
                    for p_ in pairs:
                        psc = pcps.tile([P, SBLK], F32, tag="pc", name="psc")
                        nc.tensor.matmul(
                            psc,
                            lhsT2_sb[p_],
                            qtj[:, p_, :],
                            start=True,
                            stop=True,
                        )
                        psn = pnps.tile([P, SBLK], F32, tag="pn", name="psn")
                        nc.tensor.matmul(
                            psn,
                            ksumrep_sb[p_],
                            qtj[:, p_, :],
                            start=True,
                            stop=True,
                        )
                        rcb = rcpool.tile([P, SBLK], F32, tag="rcb", name="rcb")
                        nc.vector.reciprocal_approx_fast(out=rcb[:], in_=psn[:])
                        nc.vector.tensor_tensor(
                            out=outt[:, p_, :],
                            in0=psc[:],
                            in1=rcb[:],
                            op=mybir.AluOpType.mult,
                        )

                def psc_section(j):
                    # finale-only variant: ACT-evicts psc to outu so the
                    # apply can be split per token subtile in the drain
                    qtj = qts.pop(j)
                    outu = oupool.tile([P, NPAIR, SBLK], F32, tag="outu")
                    outus[j] = outu
                    rcbs[j] = []
                    for p_ in range(NPAIR):
                        psc = pcps.tile([P, SBLK], F32, tag="pc")
                        nc.tensor.matmul(
                            psc,
                            lhsT2_sb[p_],
                            qtj[:, p_, :],
                            start=True,
                            stop=True,
                        )
                        nc.scalar.copy(out=outu[:, p_, :], in_=psc)
                        psn = pnps.tile([P, SBLK], F32, tag="pn")
                        nc.tensor.matmul(
                            psn,
                            ksumrep_sb[p_],
                            qtj[:, p_, :],
                            start=True,
                            stop=True,
                        )
                        rcb = rcpool.tile([P, SBLK], F32, tag="rcb")
                        nc.vector.reciprocal_approx_fast(out=rcb[:], in_=psn[:])
                        rcbs[j].append(rcb)

                def d_t(j, outt, t, drain=False):
                    ysb = ypool.tile([P, D], F32, tag="ysb", name="ysb")
                    psy0 = mmps.tile([P, 512], F32, tag="mm", name="psy0")
                    psy1 = mmps.tile([P, 512], F32, tag="mm", name="psy1")
                    for fs in range(FSH // P):
                        nc.tensor.matmul(
                            psy0,
                            outt[:, fs, t * P : (t + 1) * P],
                            wo_sb[:, fs, 0:512],
                            start=(fs == 0),
                            stop=(fs == FSH // P - 1),
                        )
                        nc.tensor.matmul(
                            psy1,
                            outt[:, fs, t * P : (t + 1) * P],
                            wo_sb[:, fs, 512:1024],
                            start=(fs == 0),
                            stop=(fs == FSH // P - 1),
                        )
                    # one eviction on ACT, one on DVE
                    nc.scalar.copy(out=ysb[:, 0:512], in_=psy0)
                    if drain:
                        nc.sync.dma_start(out=y_rt[j, t, 0], in_=ysb[:, 0:512])
                        nc.vector.tensor_copy(out=ysb[:, 512:1024], in_=psy1)
                        nc.sync.dma_start(out=y_rt[j, t, 1], in_=ysb[:, 512:1024])
                    else:
                        nc.vector.tensor_copy(out=ysb[:, 512:1024], in_=psy1)
                        nc.sync.dma_start(out=y_rb[j, t], in_=ysb)

                def d_half(j, ts):
                    for t in ts:
                        d_t(j, outts[j], t)

                def finale(j):
                    # drain block: apply-multiplies split per token subtile
                    # so each D chain starts as soon as its slice is scaled
                    outt = otpool.tile([P, NPAIR, SBLK], BF16, tag="outt")
                    outu = outus.pop(j)
                    rcs = rcbs.pop(j)
                    for t in range(TSUB):
                        sl = slice(t * P, (t + 1) * P)
                        for p_ in range(NPAIR):
                            nc.vector.tensor_tensor(
                                out=outt[:, p_, sl],
                                in0=outu[:, p_, sl],
                                in1=rcs[p_][:, sl],
                                op=mybir.AluOpType.mult,
                            )
                        d_t(j, outt, t, drain=(t == TSUB - 1))

                # steady-state emission: block j's Q projection brackets
                # block j-1's attention chain so the PE never waits on the
                # ACT/DVE eviction+reciprocal+apply latency; the out-proj
                # subtiles interleave between the Q halves so block
                # boundaries never wait on the last qt elu.
                for j in range(1, NBLK):
                    attn_pairs(j - 1, [0, 1])
                    attn_pairs(j - 1, [2, 3])
                    qt_half(j, 0)
                    d_half(j - 1, [0, 1])
                    qt_half(j, 1)
                    if j == NBLK - 1:
                        # last block's attention section runs ahead of the
                        # final d_t's so its evictions and reciprocals hide
                        # under D's matmuls and the drain starts immediately
                        psc_section(NBLK - 1)
                    d_half(j - 1, [2, 3])
                    outts.pop(j - 1)
                finale(NBLK - 1)

    nc.compile()
    return nc


def _prep_inputs(x, Wqkv, Wo):
    import ml_dtypes

    x = np.ascontiguousarray(x, dtype=np.float32)
    Wqkv = np.ascontiguousarray(Wqkv, dtype=np.float32)
    Wo = np.ascontiguousarray(Wo, dtype=np.float32)

    def f8(a):
        return np.clip(a, -240.0, 240.0).astype(ml_dtypes.float8_e4m3fn)

    in_maps = []
    for b in range(B):
        xT = np.ascontiguousarray(x[b].T).astype(ml_dtypes.bfloat16)  # [D, S]
        for hh in range(2):
            cols = slice(hh * FSH, (hh + 1) * FSH)
            wq = Wqkv[:, 0 * D :][:, cols]
            wk = Wqkv[:, 1 * D :][:, cols]
            wv = Wqkv[:, 2 * D :][:, cols]
            wqk8 = f8(
                np.ascontiguousarray(np.concatenate([wq, wk], axis=1)) * SW
            )
            wv_sh = np.ascontiguousarray(wv).astype(ml_dtypes.bfloat16)
            wo_sh = np.ascontiguousarray(Wo[hh * FSH : (hh + 1) * FSH, :]).astype(
                ml_dtypes.bfloat16
            )
            in_maps.append(
                {"xT": xT, "wqk8": wqk8, "wv": wv_sh, "wo": wo_sh}
            )
    return in_maps


def kernel(x, Wqkv, Wo):
    global _NC_CACHE
    if _NC_CACHE is None:
        _NC_CACHE = build()
    nc = _NC_CACHE
    in_maps = _prep_inputs(x, Wqkv, Wo)
    res = run_bass_kernel_spmd(nc, in_maps, list(range(2 * B))).results
    y = np.empty((B, S, D), dtype=np.float32)
    for b in range(B):
        y[b] = res[2 * b]["y"] + res[2 * b + 1]["y"]
    return y


# revision 19
# speedup vs baseline: 1.2267x; 1.0336x over previous
"""Linear attention (B=4, S=4096, D=1024, H=16) on 8 TRN2 NeuronCores.

Sharding: core = (batch, head-half): each core handles one batch's 8 heads.
 - x is host-transposed to xT [D, S] per batch so both operand orientations
   of every matmul come out of the tensor engine with no on-device transpose.
 - Wqkv column-sharded per head-half; Wo row-sharded; host sums the two
   partial y's per batch (row-parallel unshard).

v2: K and Q projections run in fp8e4 DoubleRow (2 contraction rows per PE
cell, ~1.8x per-matmul throughput). Errors in K/Q largely cancel through
the attention normalizer (measured end-to-end ~1.25e-2 vs 2e-2 budget);
V/out-proj paths stay bf16 (their fp8 error flows straight to the output).
Host ships x8 = e4m3(xT*16) and wqk8 = e4m3([Wq|Wk]*512); the 1/8192
descale folds into the ACT activation scale of the elu evaluation, and the
K-path relu becomes a DVE scalar_tensor_tensor multiply with a constant
tile, so ACT/DVE load is unchanged vs the bf16 kernel.

Two-phase dataflow (V/out matmuls bf16, fp32 PSUM accumulate):

phase 1 (per 512-token block): K projection fp8-DR (4 k-pair matmuls per
  128-token subtile) -> elu+1(K); V projection bf16 (8 k matmuls) ->
  [KV | K_sum^T] PSUM accumulation per head-pair (vst carries a ones
  column so one matmul does both). bf16 x is streamed per block (4-deep
  pool) since only the V projection reads it; x8 stays fully resident.
  Block 0 runs the K projection k-pair-outer (4 simultaneous PSUM chains)
  so compute starts as soon as the first (x8, wk8) DMA chunk lands.
  Block 0's Q projection runs at the end of phase 1 to bridge the
  phase transition with PE work that has no KV dependency.

phase 2 (per block, software-pipelined across j):
  QT [512f, 512s] feature-major fp8-DR (lhsT=wq8 pair, rhs=x8 pair) ->
  elu+1 -> bf16
  psc[128,s] = blockdiag(KV_h0, KV_h1)^T @ QT_pair (bf16); norm via the
  replicated-ksum matmul; 1/x on DVE fast reciprocal; outT = outu * rcp
  y[s,:] = outT^T @ Wo per 128-token subtile, fp32, one 512KB DMA per
  subtile.
"""

import numpy as np

import concourse.bacc as bacc
import concourse.mybir as mybir
import concourse.tile as tile
from concourse.bass_utils import run_bass_kernel_spmd

F32 = mybir.dt.float32
BF16 = mybir.dt.bfloat16
F8 = mybir.dt.float8e4
ACT = mybir.ActivationFunctionType
DR = mybir.MatmulPerfMode.DoubleRow

P = 128
B, S, D = 4, 4096, 1024
H = 16
HD = 64

FSH = 512            # features per core for each of Q, K, V (8 heads)
KSUB = D // P        # 8 contraction subtiles
KPAIR = KSUB // 2    # 4 fp8 DoubleRow contraction pairs
SBLK = 512           # tokens per block
NBLK = S // SBLK     # 8 blocks
TSUB = SBLK // P     # 4 token subtiles per block
NPAIR = 4            # head pairs per core

SX = 16.0            # fp8 pre-scale on x
SW = 512.0           # fp8 pre-scale on Wq/Wk
INV = 1.0 / (SX * SW)

_NC_CACHE = None


def build():
    nc = bacc.Bacc(target_bir_lowering=False)
    xT = nc.dram_tensor("xT", [D, S], BF16, kind="ExternalInput")
    wqk8 = nc.dram_tensor("wqk8", [D, 2 * FSH], F8, kind="ExternalInput")
    wv = nc.dram_tensor("wv", [D, FSH], BF16, kind="ExternalInput")
    wo = nc.dram_tensor("wo", [FSH, D], BF16, kind="ExternalInput")
    y = nc.dram_tensor("y", [S, D], F32, kind="ExternalOutput")

    xT_r = xT.rearrange("(ko p) s -> p ko s", p=P)        # [128, 8, 4096]
    wqk8_r = wqk8.rearrange("(ko p) f -> p ko f", p=P)    # [128, 8, 1024]
    wv_r = wv.rearrange("(ko p) f -> p ko f", p=P)        # [128, 8, 512]
    wo_r = wo.rearrange("(fo p) n -> p fo n", p=P)        # [128, 4, 1024]
    y_rt = y.rearrange(
        "(j t p) (nh n) -> j t nh p n", t=TSUB, p=P, nh=2
    )  # [8,4,2,128,512]
    y_rb = y.rearrange("(j t p) d -> j t p d", t=TSUB, p=P)  # [8,4,128,1024]

    with tile.TileContext(nc) as tc:
        import contextlib

        with contextlib.ExitStack() as ctx:
            wpool = ctx.enter_context(tc.tile_pool(name="wpool", bufs=1))

            # persistent SBUF
            x8_sb = wpool.tile([P, KSUB, S], F8)            # all of x8, 32KB/p
            wqk8_sb = wpool.tile([P, KSUB, 2 * FSH], F8)    # [wq8|wk8]
            wv_sb = wpool.tile([P, KSUB, FSH], BF16)
            wo_sb = wpool.tile([P, FSH // P, D], BF16)
            cinv = wpool.tile([P, SBLK], F32)               # INV const tile
            # per-pair block-diagonal [[KV_h0, 0], [0, KV_h1]] (128x128)
            lhsT2_sb = [
                wpool.tile([P, P], BF16, name=f"l2{p}") for p in range(NPAIR)
            ]
            # per-pair [ksum_h0 x64 | ksum_h1 x64] replicated along free dim
            ksumrep_sb = [
                wpool.tile([P, P], BF16, name=f"kr{p}") for p in range(NPAIR)
            ]

            xbfpool = ctx.enter_context(tc.tile_pool(name="xbf", bufs=4))
            xbfs = []

            # x8 is derived on-chip (DVE cast of the streamed bf16 x, per
            # block) instead of shipped from HBM — saves 4MB of
            # startup-critical DMA. Block 0's x arrives per k-pair
            # interleaved with the wk8 pairs so the K projection starts on
            # the first chunks; everything else queues behind on the
            # in-order sync queue.
            xbf0 = xbfpool.tile([P, KSUB, SBLK], BF16, tag="xbf", name="xbf0")
            xbfs.append(xbf0)
            for i in range(KPAIR):
                nc.sync.dma_start(
                    out=xbf0[:, 2 * i : 2 * i + 2, :],
                    in_=xT_r[:, 2 * i : 2 * i + 2, 0:SBLK],
                )
                nc.sync.dma_start(
                    out=wqk8_sb[:, 2 * i : 2 * i + 2, FSH : 2 * FSH],
                    in_=wqk8_r[:, 2 * i : 2 * i + 2, FSH : 2 * FSH],
                )
                nc.sync.dma_start(
                    out=wv_sb[:, 2 * i : 2 * i + 2, :],
                    in_=wv_r[:, 2 * i : 2 * i + 2, :],
                )

            def xbf_dma(j):
                xb = xbfpool.tile([P, KSUB, SBLK], BF16, tag="xbf", name=f"xbf{j}")
                xbfs.append(xb)
                nc.sync.dma_start(
                    out=xb, in_=xT_r[:, :, j * SBLK : (j + 1) * SBLK]
                )

            # wq8/wo are not needed until the very end of phase 1, so they
            # queue behind the first two x prefetches
            xbf_dma(1)
            nc.sync.dma_start(
                out=wqk8_sb[:, :, 0:FSH], in_=wqk8_r[:, :, 0:FSH]
            )
            xbf_dma(2)
            nc.sync.dma_start(out=wo_sb, in_=wo_r)
            for j in range(3, NBLK):
                xbf_dma(j)

            def cast_x8(j, pair=None):
                # DVE scalar-mult cast bf16 -> fp8e4 with the SX pre-scale
                sl = slice(j * SBLK, (j + 1) * SBLK)
                if pair is None:
                    nc.vector.tensor_scalar_mul(
                        out=x8_sb[:, :, sl], in0=xbfs[j], scalar1=SX
                    )
                else:
                    nc.vector.tensor_scalar_mul(
                        out=x8_sb[:, 2 * pair : 2 * pair + 2, sl],
                        in0=xbfs[j][:, 2 * pair : 2 * pair + 2, :],
                        scalar1=SX,
                    )

            nc.vector.memset(cinv, INV)
            for p_ in range(NPAIR):
                nc.vector.memset(lhsT2_sb[p_], 0.0)
                nc.vector.memset(ksumrep_sb[p_], 0.0)

            # SBUF pools shared across both phases
            etpool = ctx.enter_context(tc.tile_pool(name="et", bufs=3))
            qtpool = ctx.enter_context(tc.tile_pool(name="qt", bufs=2))
            qts = {}

            def qt_elu(ps, j, f):
                # elu(z)+1 = min(exp(z),1) + relu(z), z = ps*INV (fp8
                # descale); Exp/Relu on ACT with the scale pre-op, the
                # combine on DVE
                e = etpool.tile([P, SBLK], F32, tag="e")
                nc.scalar.activation(out=e, in_=ps, func=ACT.Exp, scale=INV)
                r = etpool.tile([P, SBLK], F32, tag="r")
                nc.scalar.activation(out=r, in_=ps, func=ACT.Relu, scale=INV)
                nc.vector.scalar_tensor_tensor(
                    out=qts[j][:, f, :],
                    in0=e,
                    scalar=1.0,
                    in1=r,
                    op0=mybir.AluOpType.min,
                    op1=mybir.AluOpType.add,
                )

            def dr_mm(ps, lhsT, rhs, i):
                nc.tensor.matmul(
                    ps,
                    lhsT,
                    rhs,
                    start=(i == 0),
                    stop=(i == KPAIR - 1),
                    perf_mode=DR,
                )

            # ---------------- phase 1: K,V projection + KV accumulation ----
            with (
                tc.tile_pool(name="kvps", bufs=1, space="PSUM") as kvps_pool,
                tc.tile_pool(name="pa", bufs=4, space="PSUM") as pa_pool,
                tc.tile_pool(name="st", bufs=2) as stpool,
            ):
                kvps = [
                    kvps_pool.tile([P, P + 1], F32, tag=f"kv{p}", name=f"kv{p}")
                    for p in range(NPAIR)
                ]

                bq = []  # lagged [KV | K_sum] accumulation entries

                def emit_b(ent):
                    kst, vst, j, t = ent
                    first = j == 0 and t == 0
                    last = j == NBLK - 1 and t == TSUB - 1
                    for p_ in range(NPAIR):
                        nc.tensor.matmul(
                            kvps[p_],
                            kst[:, t, p_ * P : (p_ + 1) * P],
                            vst[:, t, p_, :],
                            start=first,
                            stop=last,
                        )

                def elu_k(ps, kst, t):
                    # e = exp(ps*INV) on ACT; r = max(ps,0)*INV on DVE via
                    # the cinv const tile; combine min(e,1)+r on DVE
                    e = etpool.tile([P, SBLK], F32, tag="e")
                    nc.scalar.activation(out=e, in_=ps, func=ACT.Exp, scale=INV)
                    r = etpool.tile([P, SBLK], F32, tag="r")
                    nc.vector.scalar_tensor_tensor(
                        out=r,
                        in0=ps,
                        scalar=0.0,
                        in1=cinv,
                        op0=mybir.AluOpType.max,
                        op1=mybir.AluOpType.mult,
                    )
                    nc.vector.scalar_tensor_tensor(
                        out=kst[:, t, :],
                        in0=e,
                        scalar=1.0,
                        in1=r,
                        op0=mybir.AluOpType.min,
                        op1=mybir.AluOpType.add,
                    )

                # block 0: K-sweep k-pair-outer so PE work tracks DMA chunk
                # arrival (4 simultaneous PSUM chains, one per 128-token
                # subtile); V follows t-outer once wv/xbf0 have landed
                kst0 = stpool.tile([P, TSUB, FSH], BF16, tag="kst")
                vst0 = stpool.tile([P, TSUB, NPAIR, P + 1], BF16, tag="vst")
                nc.vector.memset(vst0[:, :, :, P : P + 1], 1.0)
                psks = [
                    pa_pool.tile([P, SBLK], F32, tag="pa", name=f"psk{t}")
                    for t in range(TSUB)
                ]
                for i in range(KPAIR):
                    cast_x8(0, pair=i)
                    for t in range(TSUB):
                        dr_mm(
                            psks[t],
                            x8_sb[:, 2 * i : 2 * i + 2, t * P : (t + 1) * P],
                            wqk8_sb[:, 2 * i : 2 * i + 2, FSH : 2 * FSH],
                            i,
                        )
                for t in range(TSUB):
                    elu_k(psks[t], kst0, t)
                for t in range(TSUB):
                    psv = pa_pool.tile([P, SBLK], F32, tag="pa", name=f"psv{t}")
                    for k in range(KSUB):
                        nc.tensor.matmul(
                            psv,
                            xbf0[:, k, t * P : (t + 1) * P],
                            wv_sb[:, k, :],
                            start=(k == 0),
                            stop=(k == KSUB - 1),
                        )
                    if t == 0:
                        cast_x8(1)
                    if t >= 1:
                        emit_b(bq.pop(0))
                    nc.scalar.copy(out=vst0[:, t, :, 0:P], in_=psv)
                    bq.append((kst0, vst0, 0, t))

                # blocks 1..7: token-subtile-outer, B lagged one step
                for j in range(1, NBLK):
                    kst = stpool.tile([P, TSUB, FSH], BF16, tag="kst")
                    vst = stpool.tile([P, TSUB, NPAIR, P + 1], BF16, tag="vst")
                    nc.vector.memset(vst[:, :, :, P : P + 1], 1.0)
                    xbf = xbfs[j]
                    if j + 1 < NBLK:
                        # prefetch next block's fp8 cast so it is never on
                        # the K-projection critical path
                        cast_x8(j + 1)
                    for t in range(TSUB):
                        tok = j * SBLK + t * P
                        psk = pa_pool.tile([P, SBLK], F32, tag="pa")
                        psv = pa_pool.tile([P, SBLK], F32, tag="pa")
                        for i in range(KPAIR):
                            dr_mm(
                                psk,
                                x8_sb[:, 2 * i : 2 * i + 2, tok : tok + P],
                                wqk8_sb[:, 2 * i : 2 * i + 2, FSH : 2 * FSH],
                                i,
                            )
                        for k in range(KSUB):
                            nc.tensor.matmul(
                                psv,
                                xbf[:, k, t * P : (t + 1) * P],
                                wv_sb[:, k, :],
                                start=(k == 0),
                                stop=(k == KSUB - 1),
                            )
                        emit_b(bq.pop(0))
                        elu_k(psk, kst, t)
                        nc.scalar.copy(out=vst[:, t, :, 0:P], in_=psv)
                        bq.append((kst, vst, j, t))
                # block 0's Q projection runs here, inside the phase-1 PSUM
                # pools: it has no dependency on the KV state, so it keeps
                # the PE busy across the phase boundary
                qts[0] = qtpool.tile([P, NPAIR, SBLK], BF16, tag="qt", name="qt0")
                for f in range(FSH // P):
                    psq = pa_pool.tile([P, SBLK], F32, tag="pa")
                    for i in range(KPAIR):
                        dr_mm(
                            psq,
                            wqk8_sb[:, 2 * i : 2 * i + 2, f * P : (f + 1) * P],
                            x8_sb[:, 2 * i : 2 * i + 2, 0:SBLK],
                            i,
                        )
                    if f == 0:
                        emit_b(bq.pop(0))
                        # extraction right after the final KV accumulation:
                        # the DVE copies hide under the remaining Q-chain
                        # matmuls instead of delaying phase 2's first psc
                        for p_ in range(NPAIR):
                            nc.vector.tensor_copy(
                                out=lhsT2_sb[p_][0:HD, 0:HD],
                                in_=kvps[p_][0:HD, 0:HD],
                            )
                            nc.vector.tensor_copy(
                                out=lhsT2_sb[p_][HD:P, HD:P],
                                in_=kvps[p_][HD:P, HD:P],
                            )
                            nc.vector.tensor_copy(
                                out=ksumrep_sb[p_][0:HD, 0:HD],
                                in_=kvps[p_][0:HD, P : P + 1].to_broadcast(
                                    (HD, HD)
                                ),
                            )
                            nc.vector.tensor_copy(
                                out=ksumrep_sb[p_][HD:P, HD:P],
                                in_=kvps[p_][HD:P, P : P + 1].to_broadcast(
                                    (HD, HD)
                                ),
                            )
                    qt_elu(psq, 0, f)

            # ---------------- phase 2: Q projection + attention + Wo -------
            with (
                tc.tile_pool(name="mm512", bufs=4, space="PSUM") as mmps,
                tc.tile_pool(name="pc", bufs=2, space="PSUM") as pcps,
                tc.tile_pool(name="pnb", bufs=2, space="PSUM") as pnps,
                tc.tile_pool(name="ou", bufs=3) as oupool,
                tc.tile_pool(name="rc", bufs=4) as rcpool,
                tc.tile_pool(name="ot", bufs=2) as otpool,
                tc.tile_pool(name="ys", bufs=4) as ypool,
            ):
                outus = {}
                rcbs = {}
                outts = {}

                def qt_half(j, fh):
                    if j not in qts:
                        qts[j] = qtpool.tile(
                            [P, NPAIR, SBLK], BF16, tag="qt", name=f"qt{j}"
                        )
                    for f in (2 * fh, 2 * fh + 1):
                        ps = mmps.tile([P, SBLK], F32, tag="mm")
                        for i in range(KPAIR):
                            dr_mm(
                                ps,
                                wqk8_sb[:, 2 * i : 2 * i + 2, f * P : (f + 1) * P],
                                x8_sb[:, 2 * i : 2 * i + 2, j * SBLK : (j + 1) * SBLK],
                                i,
                            )
                        qt_elu(ps, j, f)

                def attn_pairs(j, pairs):
                    # per pair: attention matmul + broadcast-normalizer
                    # matmul; the DVE apply-multiply reads psc directly from
                    # PSUM (no ACT eviction, one less latency link)
                    qtj = qts[j]
                    if j not in outts:
                        outts[j] = otpool.tile(
                            [P, NPAIR, SBLK], BF16, tag="outt", name="outt"
                        )
                    outt = outts[j]
                    for p_ in pairs:
                        psc = pcps.tile([P, SBLK], F32, tag="pc", name="psc")
                        nc.tensor.matmul(
                            psc,
                            lhsT2_sb[p_],
                            qtj[:, p_, :],
                            start=True,
                            stop=True,
                        )
                        psn = pnps.tile([P, SBLK], F32, tag="pn", name="psn")
                        nc.tensor.matmul(
                            psn,
                            ksumrep_sb[p_],
                            qtj[:, p_, :],
                            start=True,
                            stop=True,
                        )
                        rcb = rcpool.tile([P, SBLK], F32, tag="rcb", name="rcb")
                        nc.vector.reciprocal_approx_fast(out=rcb[:], in_=psn[:])
                        nc.vector.tensor_tensor(
                            out=outt[:, p_, :],
                            in0=psc[:],
                            in1=rcb[:],
                            op=mybir.AluOpType.mult,
                        )

                def psc_section(j):
                    # finale-only variant: ACT-evicts psc to outu so the
                    # apply can be split per token subtile in the drain
                    qtj = qts.pop(j)
                    outu = oupool.tile([P, NPAIR, SBLK], F32, tag="outu")
                    outus[j] = outu
                    rcbs[j] = []
                    for p_ in range(NPAIR):
                        psc = pcps.tile([P, SBLK], F32, tag="pc")
                        nc.tensor.matmul(
                            psc,
                            lhsT2_sb[p_],
                            qtj[:, p_, :],
                            start=True,
                            stop=True,
                        )
                        nc.scalar.copy(out=outu[:, p_, :], in_=psc)
                        psn = pnps.tile([P, SBLK], F32, tag="pn")
                        nc.tensor.matmul(
                            psn,
                            ksumrep_sb[p_],
                            qtj[:, p_, :],
                            start=True,
                            stop=True,
                        )
                        rcb = rcpool.tile([P, SBLK], F32, tag="rcb")
                        nc.vector.reciprocal_approx_fast(out=rcb[:], in_=psn[:])
                        rcbs[j].append(rcb)

                def d_t(j, outt, t, drain=False):
                    ysb = ypool.tile([P, D], F32, tag="ysb", name="ysb")
                    psy0 = mmps.tile([P, 512], F32, tag="mm", name="psy0")
                    psy1 = mmps.tile([P, 512], F32, tag="mm", name="psy1")
                    for fs in range(FSH // P):
                        nc.tensor.matmul(
                            psy0,
                            outt[:, fs, t * P : (t + 1) * P],
                            wo_sb[:, fs, 0:512],
                            start=(fs == 0),
                            stop=(fs == FSH // P - 1),
                        )
                        nc.tensor.matmul(
                            psy1,
                            outt[:, fs, t * P : (t + 1) * P],
                            wo_sb[:, fs, 512:1024],
                            start=(fs == 0),
                            stop=(fs == FSH // P - 1),
                        )
                    # both evictions on ACT: DVE is the congested FIFO in
                    # phase 2 (elu STT + reciprocal + apply), so PSUM-slot
                    # release must not ride it
                    nc.scalar.copy(out=ysb[:, 0:512], in_=psy0)
                    if drain:
                        nc.sync.dma_start(out=y_rt[j, t, 0], in_=ysb[:, 0:512])
                        nc.vector.tensor_copy(out=ysb[:, 512:1024], in_=psy1)
                        nc.sync.dma_start(out=y_rt[j, t, 1], in_=ysb[:, 512:1024])
                    else:
                        nc.scalar.copy(out=ysb[:, 512:1024], in_=psy1)
                        nc.sync.dma_start(out=y_rb[j, t], in_=ysb)

                def d_half(j, ts):
                    for t in ts:
                        d_t(j, outts[j], t)

                def finale(j):
                    # drain block: apply-multiplies split per token subtile
                    # so each D chain starts as soon as its slice is scaled
                    outt = otpool.tile([P, NPAIR, SBLK], BF16, tag="outt")
                    outu = outus.pop(j)
                    rcs = rcbs.pop(j)
                    for t in range(TSUB):
                        sl = slice(t * P, (t + 1) * P)
                        for p_ in range(NPAIR):
                            nc.vector.tensor_tensor(
                                out=outt[:, p_, sl],
                                in0=outu[:, p_, sl],
                                in1=rcs[p_][:, sl],
                                op=mybir.AluOpType.mult,
                            )
                        d_t(j, outt, t, drain=(t == TSUB - 1))

                # steady-state emission: block j's Q projection brackets
                # block j-1's attention chain so the PE never waits on the
                # ACT/DVE eviction+reciprocal+apply latency; the out-proj
                # subtiles interleave between the Q halves so block
                # boundaries never wait on the last qt elu.
                for j in range(1, NBLK):
                    attn_pairs(j - 1, [0, 1])
                    attn_pairs(j - 1, [2, 3])
                    qt_half(j, 0)
                    d_half(j - 1, [0, 1])
                    qt_half(j, 1)
                    if j == NBLK - 1:
                        # last block's attention section runs ahead of the
                        # final d_t's so its evictions and reciprocals hide
                        # under D's matmuls and the drain starts immediately
                        psc_section(NBLK - 1)
                    d_half(j - 1, [2, 3])
                    outts.pop(j - 1)
                finale(NBLK - 1)

    nc.compile()
    return nc


def _prep_inputs(x, Wqkv, Wo):
    import ml_dtypes

    x = np.ascontiguousarray(x, dtype=np.float32)
    Wqkv = np.ascontiguousarray(Wqkv, dtype=np.float32)
    Wo = np.ascontiguousarray(Wo, dtype=np.float32)

    def f8(a):
        return np.clip(a, -240.0, 240.0).astype(ml_dtypes.float8_e4m3fn)

    in_maps = []
    for b in range(B):
        xT = np.ascontiguousarray(x[b].T).astype(ml_dtypes.bfloat16)  # [D, S]
        for hh in range(2):
            cols = slice(hh * FSH, (hh + 1) * FSH)
            wq = Wqkv[:, 0 * D :][:, cols]
            wk = Wqkv[:, 1 * D :][:, cols]
            wv = Wqkv[:, 2 * D :][:, cols]
            wqk8 = f8(
                np.ascontiguousarray(np.concatenate([wq, wk], axis=1)) * SW
            )
            wv_sh = np.ascontiguousarray(wv).astype(ml_dtypes.bfloat16)
            wo_sh = np.ascontiguousarray(Wo[hh * FSH : (hh + 1) * FSH, :]).astype(
                ml_dtypes.bfloat16
            )
            in_maps.append(
                {"xT": xT, "wqk8": wqk8, "wv": wv_sh, "wo": wo_sh}
            )
    return in_maps


def kernel(x, Wqkv, Wo):
    global _NC_CACHE
    if _NC_CACHE is None:
        _NC_CACHE = build()
    nc = _NC_CACHE
    in_maps = _prep_inputs(x, Wqkv, Wo)
    res = run_bass_kernel_spmd(nc, in_maps, list(range(2 * B))).results
    y = np.empty((B, S, D), dtype=np.float32)
    for b in range(B):
        y[b] = res[2 * b]["y"] + res[2 * b + 1]["y"]
    return y
